# revision 4
# baseline (speedup 1.0000x reference)
"""DCRNN decoder cell (gnn_message_passing) on 8 Trainium2 cores.

Data-parallel over batch B=128 -> 16 batches/core; support matrix and
weights replicated; no collectives.

Per core (Bc=16):
  x0 = concat([xi, h]); x1 = S x0; x2 = 2 S x1 - x0     (Chebyshev)
  gate = sigmoid(sum_k Xk @ Wg_k + bg) -> r, u
  rh = r*h; y1 = S rh; y2 = 2 S y1 - rh
  c = tanh(sum_k Yk @ Wc_k + bc)
  new_h = u h + (1-u) c ; out = new_h @ pw + pb

Layouts: node-major [n, (b,u)] for Chebyshev; channel-major [u, (b,n)]
for the W-stages (out_T[o,tok] = sum_c Wk[c,o] Xk_cm[c,tok], bias and
the xi channel folded in as K=1 / K=3 matmuls).  x2_cm is computed
directly as (2 S x1 - x0)_cm via lhsT=x1_nm strips, rhs=S^T: the matmul
performs the S-apply and the cm-transpose at once.  x1_cm comes from PE
transposes of x1_nm strips.  All matmuls in float32r (full PE rate).
"""
import numpy as np

import concourse.bacc as bacc
import concourse.tile as tile
from concourse import mybir
from concourse.masks import make_identity

f32 = mybir.dt.float32
f32r = mybir.dt.float32r
AF = mybir.ActivationFunctionType
ALU = mybir.AluOpType

B, N, U, OD = 128, 1024, 128, 1
NCORES = 8
BC = B // NCORES            # 16 batches per core
KT = N // 128               # 8 node tiles
TOK = BC * N                # 16384 tokens per core
HW = BC * U                 # 2048 hidden cols
XW = HW + BC                # 2064 = hidden + xi cols

# packed small-weights tile column offsets (f32r [128, 1792])
WGH0 = 0            # [128, 3*256] gate hidden weights, c-major
WCH0 = 768          # [128, 3*128] cand hidden weights
WPW0 = WCH0 + 384   # [128, 1] proj weight
WROW = WPW0 + 1     # row-tensors region (partitions 0..2):
#   wgxi [3, 256] @ WROW, wcxi [3, 128] @ +256, gb [1, 256] @ +384 (p0),
#   cb [1,128] @ +640 (p0), pb [1,1] @ +768 (p0), ones [1,512] @ +769 (p0)
WGXI = WROW
WCXI = WROW + 256
WGB = WROW + 384
WCB = WGB + 256
WPB = WCB + 128
WONE = WPB + 1
WBIGW = WONE + 512  # 1790

_cache = {}


def _build():
    nc = bacc.Bacc("TRN2", target_bir_lowering=False, debug=False)

    s_d = nc.dram_tensor("s_t", [N, N], f32r, kind="ExternalInput").ap()
    htm_d = nc.dram_tensor("h_tm", [BC, N, U], f32r, kind="ExternalInput").ap()
    hcm_d = nc.dram_tensor("h_cm", [U, BC, N], f32r, kind="ExternalInput").ap()
    xinm_d = nc.dram_tensor("xi_nm", [N, BC], f32r, kind="ExternalInput").ap()
    xibm_d = nc.dram_tensor("xi_bm", [BC, N], f32r, kind="ExternalInput").ap()
    wgh_d = nc.dram_tensor("wgh", [3, U, 2 * U], f32r, kind="ExternalInput").ap()
    wgxi_d = nc.dram_tensor("wgxi", [3, 2 * U], f32r, kind="ExternalInput").ap()
    gb_d = nc.dram_tensor("gb", [1, 2 * U], f32r, kind="ExternalInput").ap()
    wch_d = nc.dram_tensor("wch", [3, U, U], f32r, kind="ExternalInput").ap()
    wcxi_d = nc.dram_tensor("wcxi", [3, U], f32r, kind="ExternalInput").ap()
    cb_d = nc.dram_tensor("cb", [1, U], f32r, kind="ExternalInput").ap()
    pw_d = nc.dram_tensor("pw", [U, 1], f32r, kind="ExternalInput").ap()
    pb_d = nc.dram_tensor("pb", [1, 1], f32r, kind="ExternalInput").ap()
    ones_d = nc.dram_tensor("ones", [1, 512], f32r, kind="ExternalInput").ap()

    proj_d = nc.dram_tensor("out_proj", [1, TOK], f32, kind="ExternalOutput").ap()
    nh_d = nc.dram_tensor("nh_cm", [U, BC, N], f32r, kind="ExternalOutput").ap()

    rhtm_d = nc.dram_tensor("rh_tm", [BC, N, U], f32r, kind="Internal").ap()
    rhcm_d = nc.dram_tensor("rh_cm", [U, BC, N], f32r, kind="Internal").ap()
    ucm_d = nc.dram_tensor("u_cm", [U, BC, N], f32, kind="Internal").ap()
    xist_d = nc.dram_tensor("xi_stack", [3, TOK], f32r, kind="Internal").ap()

    with tile.TileContext(nc) as tc:
        with (
            tc.tile_pool(name="consts", bufs=1) as consts,
            tc.tile_pool(name="stage1", bufs=1) as st1,
            tc.tile_pool(name="stage0", bufs=2) as st0,
            tc.tile_pool(name="pcmA", bufs=2) as pcmA,
            tc.tile_pool(name="pcmB", bufs=2) as pcmB,
            tc.tile_pool(name="pcmC", bufs=1) as pcmC,
            tc.tile_pool(name="sm2", bufs=2) as sm2,
            tc.tile_pool(name="sm1", bufs=1) as sm1,
            tc.tile_pool(name="ps", bufs=4, space="PSUM") as psAcc,
            tc.tile_pool(name="ps2", bufs=2, space="PSUM") as ps2,
        ):
            # ---------------- constants ----------------
            ident = consts.tile([128, 128], f32)
            make_identity(nc, ident)
            s_sb = consts.tile([128, KT, N], f32r)
            for kt in range(KT):
                nc.sync.dma_start(s_sb[:, kt, :], s_d[kt * 128:(kt + 1) * 128, :])
            wb = consts.tile([128, WBIGW], f32r)
            nc.sync.dma_start(
                wb[:, WGH0:WGH0 + 768].rearrange("c (k o) -> c k o", k=3),
                wgh_d[:, :, :].rearrange("k c o -> c k o"))
            nc.sync.dma_start(
                wb[:, WCH0:WCH0 + 384].rearrange("c (k o) -> c k o", k=3),
                wch_d[:, :, :].rearrange("k c o -> c k o"))
            nc.sync.dma_start(wb[:, WPW0:WPW0 + 1], pw_d[:, :])
            nc.sync.dma_start(wb[0:3, WGXI:WGXI + 256], wgxi_d[:, :])
            nc.sync.dma_start(wb[0:3, WCXI:WCXI + 128], wcxi_d[:, :])
            nc.sync.dma_start(wb[0:1, WGB:WGB + 256], gb_d[:, :])
            nc.sync.dma_start(wb[0:1, WCB:WCB + 128], cb_d[:, :])
            nc.sync.dma_start(wb[0:1, WPB:WPB + 1], pb_d[:, :])
            nc.sync.dma_start(wb[0:1, WONE:WONE + 512], ones_d[:, :])

            def wgh_k(k, o0):
                return wb[:, WGH0 + k * 256 + o0:WGH0 + k * 256 + o0 + 128]

            def wch_k(k):
                return wb[:, WCH0 + k * 128:WCH0 + (k + 1) * 128]

            ones = wb[0:1, WONE:WONE + 512]

            # ---------------- helper: node-major S-apply ----------------
            def s_apply(dst, src_dram, nb, scope):
                """dst[:, mt, :] (stage1 tile [128, KT, nb*128-wide]) =
                S @ (src_dram [nb*? ...] node-major view), streaming 256-wide
                column chunks (2 batches each)."""
                for ch in range(nb // 2):
                    b0 = 2 * ch
                    x0c = st0.tile([128, KT, 256], f32r, tag="x0h")
                    for kt in range(KT):
                        nc.sync.dma_start(
                            x0c[:, kt, :].rearrange("n (b u) -> n b u", b=2),
                            src_dram[b0:b0 + 2, kt * 128:(kt + 1) * 128, :]
                            .rearrange("b n u -> n b u"))
                    for half in range(2):
                        pss = [psAcc.tile([128, 512], f32, tag="acc",
                                          name=f"accs{half}_{mi}")
                               for mi in range(4)]
                        for kt in range(KT):
                            for mi in range(4):
                                mt = half * 4 + mi
                                nc.tensor.matmul(
                                    pss[mi][:, 0:256],
                                    s_sb[:, kt, mt * 128:(mt + 1) * 128],
                                    x0c[:, kt, :],
                                    start=(kt == 0), stop=(kt == KT - 1))
                        for mi in range(4):
                            mt = half * 4 + mi
                            nc.vector.tensor_copy(
                                dst[:, mt, ch * 256:(ch + 1) * 256],
                                pss[mi][:, 0:256])

            # ================= PHASE A: x1 = S @ x0 =================
            with nc.named_scope("phaseA"):
                x1 = st1.tile([128, KT, XW], f32r, tag="big")
                s_apply(x1, htm_d, BC, "A")
                # xi columns of x0 -> x1[:, :, HW:XW]
                x0xi = st0.tile([128, KT, BC], f32r, tag="x0xi")
                for kt in range(KT):
                    nc.sync.dma_start(x0xi[:, kt, :],
                                      xinm_d[kt * 128:(kt + 1) * 128, :])
                for half in range(2):
                    pss = [psAcc.tile([128, 512], f32, tag="acc",
                                      name=f"accx{half}_{mi}")
                           for mi in range(4)]
                    for kt in range(KT):
                        for mi in range(4):
                            mt = half * 4 + mi
                            nc.tensor.matmul(
                                pss[mi][:, 0:BC],
                                s_sb[:, kt, mt * 128:(mt + 1) * 128],
                                x0xi[:, kt, :],
                                start=(kt == 0), stop=(kt == KT - 1))
                    for mi in range(4):
                        mt = half * 4 + mi
                        nc.vector.tensor_copy(x1[:, mt, HW:XW],
                                              pss[mi][:, 0:BC])

                # xi chebyshev rows -> DRAM xi_stack
                with tc.tile_pool(name="xip", bufs=1) as xip:
                    xibm = xip.tile([BC, N], f32r)
                    nc.sync.dma_start(xibm[:], xibm_d[:, :])
                    nc.sync.dma_start(xist_d[0:1, :], xibm[:, :])
                    x1xi = xip.tile([BC, N], f32r)
                    x2xi = xip.tile([BC, N], f32r)
                    for ch in range(2):
                        c5 = slice(ch * 512, (ch + 1) * 512)
                        ps = ps2.tile([BC, 512], f32, tag="sml")
                        for kt in range(KT):
                            nc.tensor.matmul(ps[:], x0xi[:, kt, :],
                                             s_sb[:, kt, c5],
                                             start=(kt == 0), stop=(kt == KT - 1))
                        nc.vector.tensor_copy(x1xi[:, c5], ps[:])
                        psb = ps2.tile([BC, 512], f32, tag="sml")
                        for nt in range(KT):
                            nc.tensor.matmul(psb[:], x1[:, nt, HW:XW],
                                             s_sb[:, nt, c5],
                                             start=(nt == 0), stop=(nt == KT - 1))
                        nc.vector.scalar_tensor_tensor(
                            x2xi[:, c5], psb[:], 2.0, xibm[:, c5],
                            op0=ALU.mult, op1=ALU.subtract)
                    nc.sync.dma_start(xist_d[1:2, :], x1xi[:, :])
                    nc.sync.dma_start(xist_d[2:3, :], x2xi[:, :])

            # ============ PHASE B: per-batch gate pipeline ============
            with nc.named_scope("phaseB"):
                for b in range(BC):
                    bc0 = b * 128
                    hcm = pcmB.tile([U, N], f32r, tag="hcm")
                    nc.sync.dma_start(hcm[:], hcm_d[:, b, :])
                    x1cm = pcmA.tile([U, N], f32r, tag="x1cm")
                    for nt in range(KT):
                        pst = ps2.tile([128, 128], f32, tag="tr")
                        nc.tensor.transpose(
                            pst[:], x1[:, nt, bc0:bc0 + 128].bitcast(f32),
                            ident[:])
                        nc.scalar.copy(x1cm[:, nt * 128:(nt + 1) * 128], pst[:])
                    x2cm = pcmA.tile([U, N], f32r, tag="x2cm")
                    for ch in range(2):
                        c5 = slice(ch * 512, (ch + 1) * 512)
                        psx = psAcc.tile([128, 512], f32, tag="acc")
                        for nt in range(KT):
                            nc.tensor.matmul(psx[:], x1[:, nt, bc0:bc0 + 128],
                                             s_sb[:, nt, c5],
                                             start=(nt == 0), stop=(nt == KT - 1))
                        nc.vector.scalar_tensor_tensor(
                            x2cm[:, c5], psx[:], 2.0, hcm[:, c5],
                            op0=ALU.mult, op1=ALU.subtract)
                    for ch in range(2):
                        c5 = slice(ch * 512, (ch + 1) * 512)
                        t0 = b * N + ch * 512
                        xi3 = sm2.tile([3, 512], f32r, tag="xi3")
                        nc.sync.dma_start(xi3[:], xist_d[:, t0:t0 + 512])
                        for ot in range(2):
                            o0 = ot * 128
                            psw = psAcc.tile([128, 512], f32, tag="acc")
                            nc.tensor.matmul(psw[:], wgh_k(0, o0), hcm[:, c5],
                                             start=True, stop=False)
                            nc.tensor.matmul(psw[:], wgh_k(1, o0), x1cm[:, c5],
                                             start=False, stop=False)
                            nc.tensor.matmul(psw[:], wgh_k(2, o0), x2cm[:, c5],
                                             start=False, stop=False)
                            nc.tensor.matmul(psw[:],
                                             wb[0:3, WGXI + o0:WGXI + o0 + 128],
                                             xi3[:], start=False, stop=False)
                            nc.tensor.matmul(psw[:],
                                             wb[0:1, WGB + o0:WGB + o0 + 128],
                                             ones, start=False, stop=True)
                            sg = sm2.tile([128, 512], f32, tag="sg")
                            nc.scalar.activation(sg[:], psw[:], AF.Sigmoid)
                            if ot == 0:
                                rhc = sm1.tile([128, 512], f32r, tag="rhc")
                                nc.vector.tensor_mul(rhc[:], sg[:], hcm[:, c5])
                                nc.sync.dma_start(rhcm_d[:, b, c5], rhc[:])
                                for tt in range(4):
                                    m0 = ch * 512 + tt * 128
                                    pst = ps2.tile([128, 128], f32, tag="tr")
                                    nc.tensor.transpose(
                                        pst[:],
                                        rhc[:, tt * 128:(tt + 1) * 128]
                                        .bitcast(f32),
                                        ident[:])
                                    rht = sm2.tile([128, U], f32r, tag="rht")
                                    nc.vector.tensor_copy(rht[:], pst[:])
                                    nc.sync.dma_start(
                                        rhtm_d[b, m0:m0 + 128, :], rht[:])
                            else:
                                nc.sync.dma_start(ucm_d[:, b, c5], sg[:])

            # ================= PHASE C: y1 = S @ rh =================
            with nc.named_scope("phaseC"):
                y1 = st1.tile([128, KT, HW], f32r, tag="big")
                s_apply(y1, rhtm_d, BC, "C")

            # ============ PHASE D: cand + new_h + proj ============
            with nc.named_scope("phaseD"):
                for b in range(BC):
                    bc0 = b * 128
                    hcm = pcmB.tile([U, N], f32r, tag="hcm")
                    nc.sync.dma_start(hcm[:], hcm_d[:, b, :])
                    rhcm = pcmC.tile([U, N], f32r, tag="rhcm")
                    nc.sync.dma_start(rhcm[:], rhcm_d[:, b, :])
                    usb = pcmC.tile([U, N], f32, tag="usb")
                    nc.sync.dma_start(usb[:], ucm_d[:, b, :])
                    y1cm = pcmA.tile([U, N], f32r, tag="x1cm")
                    for nt in range(KT):
                        pst = ps2.tile([128, 128], f32, tag="tr")
                        nc.tensor.transpose(
                            pst[:], y1[:, nt, bc0:bc0 + 128].bitcast(f32),
                            ident[:])
                        nc.scalar.copy(y1cm[:, nt * 128:(nt + 1) * 128], pst[:])
                    y2cm = pcmA.tile([U, N], f32r, tag="x2cm")
                    for ch in range(2):
                        c5 = slice(ch * 512, (ch + 1) * 512)
                        psx = psAcc.tile([128, 512], f32, tag="acc")
                        for nt in range(KT):
                            nc.tensor.matmul(psx[:], y1[:, nt, bc0:bc0 + 128],
                                             s_sb[:, nt, c5],
                                             start=(nt == 0), stop=(nt == KT - 1))
                        nc.vector.scalar_tensor_tensor(
                            y2cm[:, c5], psx[:], 2.0, rhcm[:, c5],
                            op0=ALU.mult, op1=ALU.subtract)
                    for ch in range(2):
                        c5 = slice(ch * 512, (ch + 1) * 512)
                        t0 = b * N + ch * 512
                        xi3 = sm2.tile([3, 512], f32r, tag="xi3")
                        nc.sync.dma_start(xi3[:], xist_d[:, t0:t0 + 512])
                        psw = psAcc.tile([128, 512], f32, tag="acc")
                        nc.tensor.matmul(psw[:], wch_k(0), rhcm[:, c5],
                                         start=True, stop=False)
                        nc.tensor.matmul(psw[:], wch_k(1), y1cm[:, c5],
                                         start=False, stop=False)
                        nc.tensor.matmul(psw[:], wch_k(2), y2cm[:, c5],
                                         start=False, stop=False)
                        nc.tensor.matmul(psw[:], wb[0:3, WCXI:WCXI + 128],
                                         xi3[:], start=False, stop=False)
                        nc.tensor.matmul(psw[:], wb[0:1, WCB:WCB + 128],
                                         ones, start=False, stop=True)
                        csb = sm2.tile([128, 512], f32, tag="sg")
                        nc.scalar.activation(csb[:], psw[:], AF.Tanh)
                        # new_h = c + u*(h - c)
                        t1 = sm1.tile([128, 512], f32, tag="rhc")
                        nc.gpsimd.tensor_sub(t1[:], hcm[:, c5], csb[:])
                        t2 = sm2.tile([128, 512], f32, tag="rht")
                        nc.vector.tensor_mul(t2[:], usb[:, c5], t1[:])
                        nh = sm1.tile([128, 512], f32r, tag="nh")
                        nc.vector.tensor_add(nh[:], csb[:], t2[:])
                        nc.sync.dma_start(nh_d[:, b, c5], nh[:])
                        psp = ps2.tile([1, 512], f32, tag="sml")
                        nc.tensor.matmul(psp[:], wb[:, WPW0:WPW0 + 1], nh[:],
                                         start=True, stop=False)
                        nc.tensor.matmul(psp[:], wb[0:1, WPB:WPB + 1], ones,
                                         start=False, stop=True)
                        po = sm2.tile([1, 512], f32, tag="po")
                        nc.vector.tensor_copy(po[:], psp[:])
                        nc.sync.dma_start(proj_d[:, t0:t0 + 512], po[:])

    nc.compile()
    return nc


def _prep(inputs, hidden_state, support, gate_w, gate_b, cand_w, cand_b,
          proj_w, proj_b):
    """Host-side sharding + layout prep. Returns list of per-core in_maps."""
    f = np.float32
    S_T = np.ascontiguousarray(np.asarray(support, f).T)
    h3 = np.asarray(hidden_state, f)[0].reshape(B, N, U)
    xi = np.asarray(inputs, f).reshape(B, N)
    gw = np.asarray(gate_w, f)
    cw = np.asarray(cand_w, f)
    # W rows: flat col index c*3+k ; c=0 is xi, c=1+j is hidden channel j
    hid_rows = 3 * (1 + np.arange(U))
    shared = dict(
        s_t=S_T,
        wgh=np.ascontiguousarray(np.stack([gw[hid_rows + k] for k in range(3)])),
        wgxi=np.ascontiguousarray(gw[0:3]),
        gb=np.asarray(gate_b, f).reshape(1, 2 * U),
        wch=np.ascontiguousarray(np.stack([cw[hid_rows + k] for k in range(3)])),
        wcxi=np.ascontiguousarray(cw[0:3]),
        cb=np.asarray(cand_b, f).reshape(1, U),
        pw=np.asarray(proj_w, f).reshape(U, 1),
        pb=np.asarray(proj_b, f).reshape(1, 1),
        ones=np.ones((1, 512), f),
    )
    maps = []
    for c in range(NCORES):
        b0 = c * BC
        h_tm = np.ascontiguousarray(h3[b0:b0 + BC])               # [BC, N, U]
        h_cm = np.ascontiguousarray(h_tm.transpose(2, 0, 1))      # [U, BC, N]
        xib = np.ascontiguousarray(xi[b0:b0 + BC])                # [BC, N]
        maps.append(dict(
            shared,
            h_tm=h_tm, h_cm=h_cm,
            xi_nm=np.ascontiguousarray(xib.T),
            xi_bm=xib,
        ))
    return maps


def _gather(results):
    out = np.empty((B, N * OD), np.float32)
    new_h = np.empty((B, N * U), np.float32)
    for c in range(NCORES):
        b0 = c * BC
        r = results[c]
        out[b0:b0 + BC] = r["out_proj"].reshape(BC, N)
        new_h[b0:b0 + BC] = r["nh_cm"].transpose(1, 2, 0).reshape(BC, N * U)
    return out, new_h[None]


def kernel(**inputs):
    from concourse.bass_utils import run_bass_kernel_spmd

    if "nc" not in _cache:
        _cache["nc"] = _build()
    nc = _cache["nc"]
    maps = _prep(**inputs)
    res = run_bass_kernel_spmd(nc, maps, core_ids=list(range(NCORES)),
                               trace=False)
    _cache["last_res"] = res
    return _gather(res.results)


# revision 7
# speedup vs baseline: 1.1272x; 1.1272x over previous
"""DCRNN decoder cell (gnn_message_passing) on 8 Trainium2 cores.

Data-parallel over batch B=128 -> 16 batches/core; support matrix and
weights replicated; no collectives.

Per core (Bc=16):
  x0 = concat([xi, h]); x1 = S x0; x2 = 2 S x1 - x0     (Chebyshev)
  gate = sigmoid(sum_k Xk @ Wg_k + bg) -> r, u
  rh = r*h; y1 = S rh; y2 = 2 S y1 - rh
  c = tanh(sum_k Yk @ Wc_k + bc)
  new_h = u h + (1-u) c ; out = new_h @ pw + pb

Layouts: node-major [n, (b,u)] for Chebyshev; channel-major [u, (b,n)]
for the W-stages (out_T[o,tok] = sum_c Wk[c,o] Xk_cm[c,tok], bias and
the xi channel folded in as K=1 / K=3 matmuls).  x2_cm is computed
directly as (2 S x1 - x0)_cm via lhsT=x1_nm strips, rhs=S^T: the matmul
performs the S-apply and the cm-transpose at once.  x1_cm comes from PE
transposes of x1_nm strips.  All matmuls in float32r (full PE rate).
"""
import numpy as np

import concourse.bacc as bacc
import concourse.tile as tile
from concourse import mybir
from concourse.masks import make_identity

f32 = mybir.dt.float32
f32r = mybir.dt.float32r
AF = mybir.ActivationFunctionType
ALU = mybir.AluOpType

B, N, U, OD = 128, 1024, 128, 1
NCORES = 8
BC = B // NCORES            # 16 batches per core
KT = N // 128               # 8 node tiles
TOK = BC * N                # 16384 tokens per core
HW = BC * U                 # 2048 hidden cols
XW = HW + BC                # 2064 = hidden + xi cols

# packed small-weights tile column offsets (f32r [128, 1792])
WGH0 = 0            # [128, 3*256] gate hidden weights, c-major
WCH0 = 768          # [128, 3*128] cand hidden weights
WPW0 = WCH0 + 384   # [128, 1] proj weight
WROW = WPW0 + 1     # row-tensors region (partitions 0..2):
#   wgxi [3, 256] @ WROW, wcxi [3, 128] @ +256, gb [1, 256] @ +384 (p0),
#   cb [1,128] @ +640 (p0), pb [1,1] @ +768 (p0), ones [1,512] @ +769 (p0)
WGXI = WROW
WCXI = WROW + 256
WGBC = WCXI + 128   # [128, 2] gate bias as columns (r, u)
WCBC = WGBC + 2     # [128, 1] cand bias column
WPB = WCBC + 1      # [1, 1] proj bias
WBIGW = WPB + 1

_cache = {}


def _build():
    nc = bacc.Bacc("TRN2", target_bir_lowering=False, debug=False)

    s_d = nc.dram_tensor("s_t", [N, N], f32r, kind="ExternalInput").ap()
    htm_d = nc.dram_tensor("h_tm", [BC, N, U], f32r, kind="ExternalInput").ap()
    hcm_d = nc.dram_tensor("h_cm", [U, BC, N], f32r, kind="ExternalInput").ap()
    xinm_d = nc.dram_tensor("xi_nm", [N, BC], f32r, kind="ExternalInput").ap()
    xibm_d = nc.dram_tensor("xi_bm", [BC, N], f32r, kind="ExternalInput").ap()
    wgh_d = nc.dram_tensor("wgh", [3, U, 2 * U], f32r, kind="ExternalInput").ap()
    wgxi_d = nc.dram_tensor("wgxi", [3, 2 * U], f32r, kind="ExternalInput").ap()
    gb_d = nc.dram_tensor("gb", [1, 2 * U], f32r, kind="ExternalInput").ap()
    wch_d = nc.dram_tensor("wch", [3, U, U], f32r, kind="ExternalInput").ap()
    wcxi_d = nc.dram_tensor("wcxi", [3, U], f32r, kind="ExternalInput").ap()
    cb_d = nc.dram_tensor("cb", [1, U], f32r, kind="ExternalInput").ap()
    pw_d = nc.dram_tensor("pw", [U, 1], f32r, kind="ExternalInput").ap()
    pb_d = nc.dram_tensor("pb", [1, 1], f32r, kind="ExternalInput").ap()

    proj_d = nc.dram_tensor("out_proj", [1, TOK], f32, kind="ExternalOutput").ap()
    nh_d = nc.dram_tensor("nh_cm", [U, BC, N], f32r, kind="ExternalOutput").ap()

    x1_d = nc.dram_tensor("x1_nm", [N, XW], f32r, kind="Internal").ap()
    y1_d = nc.dram_tensor("y1_nm", [N, HW], f32r, kind="Internal").ap()
    rhtm_d = nc.dram_tensor("rh_tm", [BC, N, U], f32r, kind="Internal").ap()
    rhcm_d = nc.dram_tensor("rh_cm", [U, BC, N], f32r, kind="Internal").ap()
    ucm_d = nc.dram_tensor("u_cm", [U, BC, N], f32, kind="Internal").ap()
    xist_d = nc.dram_tensor("xi_stack", [3, TOK], f32r, kind="Internal").ap()

    with tile.TileContext(nc) as tc:
        with (
            tc.tile_pool(name="consts", bufs=1) as consts,
            tc.tile_pool(name="stage1", bufs=1) as st1,
            tc.tile_pool(name="stage0", bufs=1) as st0,
            tc.tile_pool(name="pcmA", bufs=2) as pcmA,
            tc.tile_pool(name="pcmB", bufs=2) as pcmB,
            tc.tile_pool(name="pcmC", bufs=1) as pcmC,
            tc.tile_pool(name="sm2", bufs=2) as sm2,
            tc.tile_pool(name="sm1", bufs=1) as sm1,
            tc.tile_pool(name="ps", bufs=4, space="PSUM") as psAcc,
            tc.tile_pool(name="ps2", bufs=2, space="PSUM") as ps2,
        ):
            # ---------------- constants ----------------
            ident = consts.tile([128, 128], f32)
            make_identity(nc, ident)
            s_sb = consts.tile([128, KT, N], f32r)
            for kt in range(KT):
                nc.sync.dma_start(s_sb[:, kt, :], s_d[kt * 128:(kt + 1) * 128, :])
            wb = consts.tile([128, WBIGW], f32r)
            nc.sync.dma_start(
                wb[:, WGH0:WGH0 + 768].rearrange("c (k o) -> c k o", k=3),
                wgh_d[:, :, :].rearrange("k c o -> c k o"))
            nc.sync.dma_start(
                wb[:, WCH0:WCH0 + 384].rearrange("c (k o) -> c k o", k=3),
                wch_d[:, :, :].rearrange("k c o -> c k o"))
            nc.sync.dma_start(wb[:, WPW0:WPW0 + 1], pw_d[:, :])
            nc.sync.dma_start(wb[0:3, WGXI:WGXI + 256], wgxi_d[:, :])
            nc.sync.dma_start(wb[0:3, WCXI:WCXI + 128], wcxi_d[:, :])
            nc.sync.dma_start(wb[:, WGBC:WGBC + 2],
                              gb_d[:, :].rearrange("one (t c) -> c (one t)", t=2))
            nc.sync.dma_start(wb[:, WCBC:WCBC + 1],
                              cb_d[:, :].rearrange("one c -> c one"))
            nc.sync.dma_start(wb[0:1, WPB:WPB + 1], pb_d[:, :])

            def wgh_k(k, o0):
                return wb[:, WGH0 + k * 256 + o0:WGH0 + k * 256 + o0 + 128]

            def wch_k(k):
                return wb[:, WCH0 + k * 128:WCH0 + (k + 1) * 128]

            # -------- helper: node-major S-apply, resident source --------
            def s_apply(src_res, width, dst_dram):
                """dst_dram[mt-block, :width] = S @ src_res ([128, KT, width]).
                mt/kt-outer so each stationary S-tile serves all chunks."""
                nch = width // 512
                tail = width - nch * 512
                for mt in range(KT):
                    pss = [psAcc.tile([128, 512], f32, tag="acc",
                                      name=f"apl{mt}_{ci}")
                           for ci in range(nch)]
                    pxi = (ps2.tile([128, 128], f32, tag="tr",
                                    name=f"apx{mt}")
                           if tail else None)
                    for kt in range(KT):
                        for ci in range(nch):
                            nc.tensor.matmul(
                                pss[ci][:], s_sb[:, kt, mt * 128:(mt + 1) * 128],
                                src_res[:, kt, ci * 512:(ci + 1) * 512],
                                start=(kt == 0), stop=(kt == KT - 1))
                        if tail:
                            nc.tensor.matmul(
                                pxi[:, 0:tail],
                                s_sb[:, kt, mt * 128:(mt + 1) * 128],
                                src_res[:, kt, nch * 512:width],
                                start=(kt == 0), stop=(kt == KT - 1))
                    for ci in range(nch):
                        so = sm2.tile([128, 512], f32r, tag="sg",
                                      name=f"so{mt}_{ci}")
                        nc.vector.tensor_copy(so[:], pss[ci][:])
                        nc.sync.dma_start(
                            dst_dram[mt * 128:(mt + 1) * 128,
                                     ci * 512:(ci + 1) * 512], so[:])
                    if tail:
                        sot = sm2.tile([128, tail], f32r, tag="po",
                                       name=f"sot{mt}")
                        nc.vector.tensor_copy(sot[:], pxi[:, 0:tail])
                        nc.sync.dma_start(
                            dst_dram[mt * 128:(mt + 1) * 128, nch * 512:width],
                            sot[:])

            # ================= PHASE A: x1 = S @ x0 =================
            with nc.named_scope("phaseA"):
                x0 = st0.tile([128, KT, XW], f32r, tag="big")
                for kt in range(KT):
                    nc.sync.dma_start(
                        x0[:, kt, 0:HW].rearrange("n (b u) -> n b u", b=BC),
                        htm_d[:, kt * 128:(kt + 1) * 128, :]
                        .rearrange("b n u -> n b u"))
                    nc.sync.dma_start(x0[:, kt, HW:XW],
                                      xinm_d[kt * 128:(kt + 1) * 128, :])
                s_apply(x0, XW, x1_d)

                # xi chebyshev rows -> DRAM xi_stack
                with tc.tile_pool(name="xip", bufs=1) as xip:
                    xibm = xip.tile([BC, N], f32r)
                    nc.sync.dma_start(xibm[:], xibm_d[:, :])
                    nc.sync.dma_start(xist_d[0:1, :], xibm[:, :])
                    x1xit = xip.tile([128, KT, BC], f32r)
                    nc.sync.dma_start(
                        x1xit[:], x1_d[:, HW:XW].rearrange("(k n) c -> n k c",
                                                           n=128))
                    x1xi = xip.tile([BC, N], f32r)
                    x2xi = xip.tile([BC, N], f32r)
                    for ch in range(2):
                        c5 = slice(ch * 512, (ch + 1) * 512)
                        ps = ps2.tile([BC, 512], f32, tag="sml")
                        for kt in range(KT):
                            nc.tensor.matmul(ps[:], x0[:, kt, HW:XW],
                                             s_sb[:, kt, c5],
                                             start=(kt == 0), stop=(kt == KT - 1))
                        nc.vector.tensor_copy(x1xi[:, c5], ps[:])
                        psb = ps2.tile([BC, 512], f32, tag="sml")
                        for nt in range(KT):
                            nc.tensor.matmul(psb[:], x1xit[:, nt, :],
                                             s_sb[:, nt, c5],
                                             start=(nt == 0), stop=(nt == KT - 1))
                        nc.vector.scalar_tensor_tensor(
                            x2xi[:, c5], psb[:], 2.0, xibm[:, c5],
                            op0=ALU.mult, op1=ALU.subtract)
                    nc.sync.dma_start(xist_d[1:2, :], x1xi[:, :])
                    nc.sync.dma_start(xist_d[2:3, :], x2xi[:, :])

            # ============ PHASE B: per-batch gate pipeline ============
            with nc.named_scope("phaseB"):
                for b in range(BC):
                    bc0 = b * 128
                    hcm = pcmB.tile([U, N], f32r, tag="hcm")
                    nc.sync.dma_start(hcm[:], hcm_d[:, b, :])
                    x1t = pcmA.tile([128, KT, 128], f32r, tag="x1t")
                    nc.sync.dma_start(
                        x1t[:], x1_d[:, bc0:bc0 + 128]
                        .rearrange("(k n) c -> n k c", n=128))
                    xi3 = sm2.tile([3, N], f32r, tag="xi3")
                    nc.sync.dma_start(xi3[:], xist_d[:, b * N:(b + 1) * N])
                    x1cm = pcmA.tile([U, N], f32r, tag="x1cm")
                    for nt in range(KT):
                        pst = ps2.tile([128, 128], f32, tag="tr")
                        nc.tensor.transpose(
                            pst[:], x1t[:, nt, :].bitcast(f32), ident[:])
                        nc.scalar.copy(x1cm[:, nt * 128:(nt + 1) * 128], pst[:])
                    x2cm = pcmA.tile([U, N], f32r, tag="x2cm")
                    psx = [psAcc.tile([128, 512], f32, tag="acc",
                                      name=f"bx{b}_{ci}") for ci in range(2)]
                    for nt in range(KT):
                        for ci in range(2):
                            nc.tensor.matmul(
                                psx[ci][:], x1t[:, nt, :],
                                s_sb[:, nt, ci * 512:(ci + 1) * 512],
                                start=(nt == 0), stop=(nt == KT - 1))
                    for ci in range(2):
                        c5 = slice(ci * 512, (ci + 1) * 512)
                        nc.vector.scalar_tensor_tensor(
                            x2cm[:, c5], psx[ci][:], 2.0, hcm[:, c5],
                            op0=ALU.mult, op1=ALU.subtract)
                    # gate W-stage: 4 psums (2 ot x 2 ch), k-outer
                    psw = [[psAcc.tile([128, 512], f32, tag="acc",
                                       name=f"bw{b}_{ot}_{ci}")
                            for ci in range(2)] for ot in range(2)]
                    rhs_k = [hcm, x1cm, x2cm]
                    for ot in range(2):
                        o0 = ot * 128
                        for k in range(3):
                            for ci in range(2):
                                c5 = slice(ci * 512, (ci + 1) * 512)
                                nc.tensor.matmul(psw[ot][ci][:], wgh_k(k, o0),
                                                 rhs_k[k][:, c5],
                                                 start=(k == 0), stop=False)
                        for ci in range(2):
                            c5 = slice(ci * 512, (ci + 1) * 512)
                            nc.tensor.matmul(psw[ot][ci][:],
                                             wb[0:3, WGXI + o0:WGXI + o0 + 128],
                                             xi3[:, c5], start=False, stop=True)
                    for ci in range(2):
                        c5 = slice(ci * 512, (ci + 1) * 512)
                        # r
                        sgr = sm2.tile([128, 512], f32, tag="sg")
                        nc.scalar.activation(sgr[:], psw[0][ci][:], AF.Sigmoid,
                                             bias=wb[:, WGBC:WGBC + 1])
                        rhc = sm1.tile([128, 512], f32r, tag="rhc")
                        nc.vector.tensor_mul(rhc[:], sgr[:], hcm[:, c5])
                        nc.sync.dma_start(rhcm_d[:, b, c5], rhc[:])
                        rht = sm2.tile([128, 4, U], f32r, tag="rht")
                        for tt in range(4):
                            pst = ps2.tile([128, 128], f32, tag="tr")
                            nc.tensor.transpose(
                                pst[:], rhc[:, tt * 128:(tt + 1) * 128]
                                .bitcast(f32), ident[:])
                            nc.vector.tensor_copy(rht[:, tt, :], pst[:])
                        m0 = ci * 512
                        nc.sync.dma_start(
                            rhtm_d[b, m0:m0 + 512, :]
                            .rearrange("(t n) u -> n t u", t=4), rht[:])
                        # u
                        sgu = sm2.tile([128, 512], f32, tag="sg")
                        nc.scalar.activation(sgu[:], psw[1][ci][:], AF.Sigmoid,
                                             bias=wb[:, WGBC + 1:WGBC + 2])
                        nc.sync.dma_start(ucm_d[:, b, c5], sgu[:])

            # ================= PHASE C: y1 = S @ rh =================
            with nc.named_scope("phaseC"):
                rh0 = st0.tile([128, KT, HW], f32r, tag="big")
                for kt in range(KT):
                    nc.sync.dma_start(
                        rh0[:, kt, :].rearrange("n (b u) -> n b u", b=BC),
                        rhtm_d[:, kt * 128:(kt + 1) * 128, :]
                        .rearrange("b n u -> n b u"))
                s_apply(rh0, HW, y1_d)

            # ============ PHASE D: cand + new_h + proj ============
            with nc.named_scope("phaseD"):
                for b in range(BC):
                    bc0 = b * 128
                    hcm = pcmB.tile([U, N], f32r, tag="hcm")
                    nc.sync.dma_start(hcm[:], hcm_d[:, b, :])
                    rhcm = pcmC.tile([U, N], f32r, tag="rhcm")
                    nc.sync.dma_start(rhcm[:], rhcm_d[:, b, :])
                    usb = pcmC.tile([U, N], f32, tag="usb")
                    nc.sync.dma_start(usb[:], ucm_d[:, b, :])
                    y1t = pcmA.tile([128, KT, 128], f32r, tag="x1t")
                    nc.sync.dma_start(
                        y1t[:], y1_d[:, bc0:bc0 + 128]
                        .rearrange("(k n) c -> n k c", n=128))
                    xi3 = sm2.tile([3, N], f32r, tag="xi3")
                    nc.sync.dma_start(xi3[:], xist_d[:, b * N:(b + 1) * N])
                    y1cm = pcmA.tile([U, N], f32r, tag="x1cm")
                    for nt in range(KT):
                        pst = ps2.tile([128, 128], f32, tag="tr")
                        nc.tensor.transpose(
                            pst[:], y1t[:, nt, :].bitcast(f32), ident[:])
                        nc.scalar.copy(y1cm[:, nt * 128:(nt + 1) * 128], pst[:])
                    y2cm = pcmA.tile([U, N], f32r, tag="x2cm")
                    psx = [psAcc.tile([128, 512], f32, tag="acc",
                                      name=f"dx{b}_{ci}") for ci in range(2)]
                    for nt in range(KT):
                        for ci in range(2):
                            nc.tensor.matmul(
                                psx[ci][:], y1t[:, nt, :],
                                s_sb[:, nt, ci * 512:(ci + 1) * 512],
                                start=(nt == 0), stop=(nt == KT - 1))
                    for ci in range(2):
                        c5 = slice(ci * 512, (ci + 1) * 512)
                        nc.vector.scalar_tensor_tensor(
                            y2cm[:, c5], psx[ci][:], 2.0, rhcm[:, c5],
                            op0=ALU.mult, op1=ALU.subtract)
                    psw = [psAcc.tile([128, 512], f32, tag="acc",
                                      name=f"dw{b}_{ci}") for ci in range(2)]
                    rhs_k = [rhcm, y1cm, y2cm]
                    for k in range(3):
                        for ci in range(2):
                            c5 = slice(ci * 512, (ci + 1) * 512)
                            nc.tensor.matmul(psw[ci][:], wch_k(k),
                                             rhs_k[k][:, c5],
                                             start=(k == 0), stop=False)
                    for ci in range(2):
                        c5 = slice(ci * 512, (ci + 1) * 512)
                        nc.tensor.matmul(psw[ci][:], wb[0:3, WCXI:WCXI + 128],
                                         xi3[:, c5], start=False, stop=True)
                    for ci in range(2):
                        c5 = slice(ci * 512, (ci + 1) * 512)
                        t0 = b * N + ci * 512
                        csb = sm2.tile([128, 512], f32, tag="sg")
                        nc.scalar.activation(csb[:], psw[ci][:], AF.Tanh,
                                             bias=wb[:, WCBC:WCBC + 1])
                        # new_h = c + u*(h - c)
                        t1 = sm1.tile([128, 512], f32, tag="rhc")
                        nc.vector.tensor_sub(t1[:], hcm[:, c5], csb[:])
                        t2 = sm2.tile([128, 512], f32, tag="t2")
                        nc.vector.tensor_mul(t2[:], usb[:, c5], t1[:])
                        nh = sm1.tile([128, 512], f32r, tag="nh")
                        nc.vector.tensor_add(nh[:], csb[:], t2[:])
                        nc.sync.dma_start(nh_d[:, b, c5], nh[:])
                        psp = ps2.tile([1, 512], f32, tag="sml")
                        nc.tensor.matmul(psp[:], wb[:, WPW0:WPW0 + 1], nh[:],
                                         start=True, stop=True)
                        po = sm2.tile([1, 512], f32, tag="po")
                        nc.scalar.activation(po[:], psp[:], AF.Identity,
                                             bias=wb[0:1, WPB:WPB + 1])
                        nc.sync.dma_start(proj_d[:, t0:t0 + 512], po[:])

    nc.compile()
    return nc


def _prep(inputs, hidden_state, support, gate_w, gate_b, cand_w, cand_b,
          proj_w, proj_b):
    """Host-side sharding + layout prep. Returns list of per-core in_maps."""
    f = np.float32
    S_T = np.ascontiguousarray(np.asarray(support, f).T)
    h3 = np.asarray(hidden_state, f)[0].reshape(B, N, U)
    xi = np.asarray(inputs, f).reshape(B, N)
    gw = np.asarray(gate_w, f)
    cw = np.asarray(cand_w, f)
    # W rows: flat col index c*3+k ; c=0 is xi, c=1+j is hidden channel j
    hid_rows = 3 * (1 + np.arange(U))
    shared = dict(
        s_t=S_T,
        wgh=np.ascontiguousarray(np.stack([gw[hid_rows + k] for k in range(3)])),
        wgxi=np.ascontiguousarray(gw[0:3]),
        gb=np.asarray(gate_b, f).reshape(1, 2 * U),
        wch=np.ascontiguousarray(np.stack([cw[hid_rows + k] for k in range(3)])),
        wcxi=np.ascontiguousarray(cw[0:3]),
        cb=np.asarray(cand_b, f).reshape(1, U),
        pw=np.asarray(proj_w, f).reshape(U, 1),
        pb=np.asarray(proj_b, f).reshape(1, 1),
    )
    maps = []
    for c in range(NCORES):
        b0 = c * BC
        h_tm = np.ascontiguousarray(h3[b0:b0 + BC])               # [BC, N, U]
        h_cm = np.ascontiguousarray(h_tm.transpose(2, 0, 1))      # [U, BC, N]
        xib = np.ascontiguousarray(xi[b0:b0 + BC])                # [BC, N]
        maps.append(dict(
            shared,
            h_tm=h_tm, h_cm=h_cm,
            xi_nm=np.ascontiguousarray(xib.T),
            xi_bm=xib,
        ))
    return maps


def _gather(results):
    out = np.empty((B, N * OD), np.float32)
    new_h = np.empty((B, N * U), np.float32)
    for c in range(NCORES):
        b0 = c * BC
        r = results[c]
        out[b0:b0 + BC] = r["out_proj"].reshape(BC, N)
        new_h[b0:b0 + BC] = r["nh_cm"].transpose(1, 2, 0).reshape(BC, N * U)
    return out, new_h[None]


def kernel(**inputs):
    from concourse.bass_utils import run_bass_kernel_spmd

    if "nc" not in _cache:
        _cache["nc"] = _build()
    nc = _cache["nc"]
    maps = _prep(**inputs)
    res = run_bass_kernel_spmd(nc, maps, core_ids=list(range(NCORES)),
                               trace=False)
    _cache["last_res"] = res
    return _gather(res.results)


# revision 8
# speedup vs baseline: 1.3008x; 1.1541x over previous
"""DCRNN decoder cell (gnn_message_passing) on 8 Trainium2 cores.

Data-parallel over batch B=128 -> 16 batches/core; support matrix and
weights replicated; no collectives.

Per core (Bc=16):
  x0 = concat([xi, h]); x1 = S x0; x2 = 2 S x1 - x0     (Chebyshev)
  gate = sigmoid(sum_k Xk @ Wg_k + bg) -> r, u
  rh = r*h; y1 = S rh; y2 = 2 S y1 - rh
  c = tanh(sum_k Yk @ Wc_k + bc)
  new_h = u h + (1-u) c ; out = new_h @ pw + pb

Layouts: node-major [n, (b,u)] for Chebyshev; channel-major [u, (b,n)]
for the W-stages (out_T[o,tok] = sum_c Wk[c,o] Xk_cm[c,tok], bias and
the xi channel folded in as K=1 / K=3 matmuls).  x2_cm is computed
directly as (2 S x1 - x0)_cm via lhsT=x1_nm strips, rhs=S^T: the matmul
performs the S-apply and the cm-transpose at once.  x1_cm comes from PE
transposes of x1_nm strips.  All matmuls in float32r (full PE rate).
"""
import numpy as np

import concourse.bacc as bacc
import concourse.tile as tile
from concourse import mybir
from concourse.masks import make_identity

f32 = mybir.dt.float32
f32r = mybir.dt.float32r
AF = mybir.ActivationFunctionType
ALU = mybir.AluOpType

B, N, U, OD = 128, 1024, 128, 1
NCORES = 8
BC = B // NCORES            # 16 batches per core
KT = N // 128               # 8 node tiles
TOK = BC * N                # 16384 tokens per core
HW = BC * U                 # 2048 hidden cols
XW = HW + BC                # 2064 = hidden + xi cols

# packed small-weights tile column offsets (f32r [128, 1792])
WGH0 = 0            # [128, 3*256] gate hidden weights, c-major
WCH0 = 768          # [128, 3*128] cand hidden weights
WPW0 = WCH0 + 384   # [128, 1] proj weight
WROW = WPW0 + 1     # row-tensors region (partitions 0..2):
#   wgxi [3, 256] @ WROW, wcxi [3, 128] @ +256, gb [1, 256] @ +384 (p0),
#   cb [1,128] @ +640 (p0), pb [1,1] @ +768 (p0), ones [1,512] @ +769 (p0)
WGXI = WROW
WCXI = WROW + 256
WGBC = WCXI + 128   # [128, 2] gate bias as columns (r, u)
WCBC = WGBC + 2     # [128, 1] cand bias column
WPB = WCBC + 1      # [1, 1] proj bias
WBIGW = WPB + 1

_cache = {}


def _build():
    nc = bacc.Bacc("TRN2", target_bir_lowering=False, debug=False)

    s_d = nc.dram_tensor("s_t", [N, N], f32r, kind="ExternalInput").ap()
    htm_d = nc.dram_tensor("h_tm", [BC, N, U], f32r, kind="ExternalInput").ap()
    hcm_d = nc.dram_tensor("h_cm", [U, BC, N], f32r, kind="ExternalInput").ap()
    xinm_d = nc.dram_tensor("xi_nm", [N, BC], f32r, kind="ExternalInput").ap()
    xibm_d = nc.dram_tensor("xi_bm", [BC, N], f32r, kind="ExternalInput").ap()
    wgh_d = nc.dram_tensor("wgh", [3, U, 2 * U], f32r, kind="ExternalInput").ap()
    wgxi_d = nc.dram_tensor("wgxi", [3, 2 * U], f32r, kind="ExternalInput").ap()
    gb_d = nc.dram_tensor("gb", [1, 2 * U], f32r, kind="ExternalInput").ap()
    wch_d = nc.dram_tensor("wch", [3, U, U], f32r, kind="ExternalInput").ap()
    wcxi_d = nc.dram_tensor("wcxi", [3, U], f32r, kind="ExternalInput").ap()
    cb_d = nc.dram_tensor("cb", [1, U], f32r, kind="ExternalInput").ap()
    pw_d = nc.dram_tensor("pw", [U, 1], f32r, kind="ExternalInput").ap()
    pb_d = nc.dram_tensor("pb", [1, 1], f32r, kind="ExternalInput").ap()

    proj_d = nc.dram_tensor("out_proj", [1, TOK], f32, kind="ExternalOutput").ap()
    nh_d = nc.dram_tensor("nh_cm", [U, BC, N], f32r, kind="ExternalOutput").ap()

    x1_d = nc.dram_tensor("x1_nm", [N, XW], f32r, kind="Internal").ap()
    y1_d = nc.dram_tensor("y1_nm", [N, HW], f32r, kind="Internal").ap()
    rhtm_d = nc.dram_tensor("rh_tm", [BC, N, U], f32r, kind="Internal").ap()
    rhcm_d = nc.dram_tensor("rh_cm", [U, BC, N], f32r, kind="Internal").ap()
    ucm_d = nc.dram_tensor("u_cm", [U, BC, N], f32, kind="Internal").ap()
    xist_d = nc.dram_tensor("xi_stack", [3, TOK], f32r, kind="Internal").ap()

    with tile.TileContext(nc) as tc:
        with (
            tc.tile_pool(name="consts", bufs=1) as consts,
            tc.tile_pool(name="stage1", bufs=1) as st1,
            tc.tile_pool(name="stage0", bufs=1) as st0,
            tc.tile_pool(name="pcmA", bufs=2) as pcmA,
            tc.tile_pool(name="pcmB", bufs=2) as pcmB,
            tc.tile_pool(name="pcmC", bufs=2) as pcmC,
            tc.tile_pool(name="sm2", bufs=2) as sm2,
            tc.tile_pool(name="sm1", bufs=2) as sm1,
            tc.tile_pool(name="ps", bufs=4, space="PSUM") as psAcc,
            tc.tile_pool(name="ps2", bufs=2, space="PSUM") as ps2,
        ):
            # ---------------- constants ----------------
            ident = consts.tile([128, 128], f32)
            make_identity(nc, ident)
            s_sb = consts.tile([128, KT, N], f32r)
            for kt in range(KT):
                nc.sync.dma_start(s_sb[:, kt, :], s_d[kt * 128:(kt + 1) * 128, :])
            wb = consts.tile([128, WBIGW], f32r)
            nc.sync.dma_start(
                wb[:, WGH0:WGH0 + 768].rearrange("c (k o) -> c k o", k=3),
                wgh_d[:, :, :].rearrange("k c o -> c k o"))
            nc.sync.dma_start(
                wb[:, WCH0:WCH0 + 384].rearrange("c (k o) -> c k o", k=3),
                wch_d[:, :, :].rearrange("k c o -> c k o"))
            nc.sync.dma_start(wb[:, WPW0:WPW0 + 1], pw_d[:, :])
            nc.sync.dma_start(wb[0:3, WGXI:WGXI + 256], wgxi_d[:, :])
            nc.sync.dma_start(wb[0:3, WCXI:WCXI + 128], wcxi_d[:, :])
            nc.sync.dma_start(wb[:, WGBC:WGBC + 2],
                              gb_d[:, :].rearrange("one (t c) -> c (one t)", t=2))
            nc.sync.dma_start(wb[:, WCBC:WCBC + 1],
                              cb_d[:, :].rearrange("one c -> c one"))
            nc.sync.dma_start(wb[0:1, WPB:WPB + 1], pb_d[:, :])

            def wgh_k(k, o0):
                return wb[:, WGH0 + k * 256 + o0:WGH0 + k * 256 + o0 + 128]

            def wch_k(k):
                return wb[:, WCH0 + k * 128:WCH0 + (k + 1) * 128]

            # -------- helper: node-major S-apply, resident source --------
            def s_apply(src_res, width, dst_dram):
                """dst_dram[mt-block, :width] = S @ src_res ([128, KT, width]).
                mt/kt-outer so each stationary S-tile serves all chunks."""
                nch = width // 512
                tail = width - nch * 512
                for mt in range(KT):
                    pss = [psAcc.tile([128, 512], f32, tag="acc",
                                      name=f"apl{mt}_{ci}")
                           for ci in range(nch)]
                    pxi = (ps2.tile([128, 128], f32, tag="tr",
                                    name=f"apx{mt}")
                           if tail else None)
                    for kt in range(KT):
                        for ci in range(nch):
                            nc.tensor.matmul(
                                pss[ci][:], s_sb[:, kt, mt * 128:(mt + 1) * 128],
                                src_res[:, kt, ci * 512:(ci + 1) * 512],
                                start=(kt == 0), stop=(kt == KT - 1))
                        if tail:
                            nc.tensor.matmul(
                                pxi[:, 0:tail],
                                s_sb[:, kt, mt * 128:(mt + 1) * 128],
                                src_res[:, kt, nch * 512:width],
                                start=(kt == 0), stop=(kt == KT - 1))
                    for ci in range(nch):
                        so = sm2.tile([128, 512], f32r, tag="sg",
                                      name=f"so{mt}_{ci}")
                        nc.vector.tensor_copy(so[:], pss[ci][:])
                        nc.sync.dma_start(
                            dst_dram[mt * 128:(mt + 1) * 128,
                                     ci * 512:(ci + 1) * 512], so[:])
                    if tail:
                        sot = sm2.tile([128, tail], f32r, tag="po",
                                       name=f"sot{mt}")
                        nc.vector.tensor_copy(sot[:], pxi[:, 0:tail])
                        nc.sync.dma_start(
                            dst_dram[mt * 128:(mt + 1) * 128, nch * 512:width],
                            sot[:])

            # ================= PHASE A: x1 = S @ x0 =================
            with nc.named_scope("phaseA"):
                x0 = st0.tile([128, KT, XW], f32r, tag="big")
                for kt in range(KT):
                    nc.sync.dma_start(
                        x0[:, kt, 0:HW].rearrange("n (b u) -> n b u", b=BC),
                        htm_d[:, kt * 128:(kt + 1) * 128, :]
                        .rearrange("b n u -> n b u"))
                    nc.sync.dma_start(x0[:, kt, HW:XW],
                                      xinm_d[kt * 128:(kt + 1) * 128, :])
                s_apply(x0, XW, x1_d)

                # xi chebyshev rows -> DRAM xi_stack
                with tc.tile_pool(name="xip", bufs=1) as xip:
                    xibm = xip.tile([BC, N], f32r)
                    nc.sync.dma_start(xibm[:], xibm_d[:, :])
                    nc.sync.dma_start(xist_d[0:1, :], xibm[:, :])
                    x1xit = xip.tile([128, KT, BC], f32r)
                    nc.sync.dma_start(
                        x1xit[:], x1_d[:, HW:XW].rearrange("(k n) c -> n k c",
                                                           n=128))
                    x1xi = xip.tile([BC, N], f32r)
                    x2xi = xip.tile([BC, N], f32r)
                    for ch in range(2):
                        c5 = slice(ch * 512, (ch + 1) * 512)
                        ps = ps2.tile([BC, 512], f32, tag="sml")
                        for kt in range(KT):
                            nc.tensor.matmul(ps[:], x0[:, kt, HW:XW],
                                             s_sb[:, kt, c5],
                                             start=(kt == 0), stop=(kt == KT - 1))
                        nc.vector.tensor_copy(x1xi[:, c5], ps[:])
                        psb = ps2.tile([BC, 512], f32, tag="sml")
                        for nt in range(KT):
                            nc.tensor.matmul(psb[:], x1xit[:, nt, :],
                                             s_sb[:, nt, c5],
                                             start=(nt == 0), stop=(nt == KT - 1))
                        nc.vector.scalar_tensor_tensor(
                            x2xi[:, c5], psb[:], 2.0, xibm[:, c5],
                            op0=ALU.mult, op1=ALU.subtract)
                    nc.sync.dma_start(xist_d[1:2, :], x1xi[:, :])
                    nc.sync.dma_start(xist_d[2:3, :], x2xi[:, :])

            # ============ PHASE B: per-batch gate pipeline ============
            with nc.named_scope("phaseB"):
                for b in range(BC):
                    bc0 = b * 128
                    hcm = pcmB.tile([U, N], f32r, tag="hcm")
                    nc.sync.dma_start(hcm[:], hcm_d[:, b, :])
                    x1t = pcmA.tile([128, KT, 128], f32r, tag="x1t")
                    nc.sync.dma_start(
                        x1t[:], x1_d[:, bc0:bc0 + 128]
                        .rearrange("(k n) c -> n k c", n=128))
                    xi3 = sm2.tile([3, N], f32r, tag="xi3")
                    nc.sync.dma_start(xi3[:], xist_d[:, b * N:(b + 1) * N])
                    x1cm = pcmA.tile([U, N], f32r, tag="x1cm")
                    for np2 in range(KT // 2):
                        pst = ps2.tile([128, 256], f32, tag="tr")
                        for j in range(2):
                            nc.tensor.transpose(
                                pst[:, j * 128:(j + 1) * 128],
                                x1t[:, np2 * 2 + j, :].bitcast(f32), ident[:])
                        nc.scalar.copy(x1cm[:, np2 * 256:(np2 + 1) * 256], pst[:])
                    x2cm = pcmA.tile([U, N], f32r, tag="x2cm")
                    psx = [psAcc.tile([128, 512], f32, tag="acc",
                                      name=f"bx{b}_{ci}") for ci in range(2)]
                    for nt in range(KT):
                        for ci in range(2):
                            nc.tensor.matmul(
                                psx[ci][:], x1t[:, nt, :],
                                s_sb[:, nt, ci * 512:(ci + 1) * 512],
                                start=(nt == 0), stop=(nt == KT - 1))
                    for ci in range(2):
                        c5 = slice(ci * 512, (ci + 1) * 512)
                        nc.vector.scalar_tensor_tensor(
                            x2cm[:, c5], psx[ci][:], 2.0, hcm[:, c5],
                            op0=ALU.mult, op1=ALU.subtract)
                    # gate W-stage: 4 psums (2 ot x 2 ch), k-outer
                    psw = [[psAcc.tile([128, 512], f32, tag="acc",
                                       name=f"bw{b}_{ot}_{ci}")
                            for ci in range(2)] for ot in range(2)]
                    rhs_k = [hcm, x1cm, x2cm]
                    for ot in range(2):
                        o0 = ot * 128
                        for k in range(3):
                            for ci in range(2):
                                c5 = slice(ci * 512, (ci + 1) * 512)
                                nc.tensor.matmul(psw[ot][ci][:], wgh_k(k, o0),
                                                 rhs_k[k][:, c5],
                                                 start=(k == 0), stop=False)
                        for ci in range(2):
                            c5 = slice(ci * 512, (ci + 1) * 512)
                            nc.tensor.matmul(psw[ot][ci][:],
                                             wb[0:3, WGXI + o0:WGXI + o0 + 128],
                                             xi3[:, c5], start=False, stop=True)
                    for ci in range(2):
                        c5 = slice(ci * 512, (ci + 1) * 512)
                        # r
                        sgr = sm2.tile([128, 512], f32, tag="sg")
                        nc.scalar.activation(sgr[:], psw[0][ci][:], AF.Sigmoid,
                                             bias=wb[:, WGBC:WGBC + 1])
                        rhc = sm1.tile([128, 512], f32r, tag="rhc")
                        nc.vector.tensor_mul(rhc[:], sgr[:], hcm[:, c5])
                        nc.gpsimd.dma_start(rhcm_d[:, b, c5], rhc[:])
                        rht = sm2.tile([128, 4, U], f32r, tag="rht")
                        for tp in range(2):
                            pst = ps2.tile([128, 256], f32, tag="tr")
                            for j in range(2):
                                nc.tensor.transpose(
                                    pst[:, j * 128:(j + 1) * 128],
                                    rhc[:, (tp * 2 + j) * 128:
                                        (tp * 2 + j + 1) * 128]
                                    .bitcast(f32), ident[:])
                            nc.vector.tensor_copy(
                                rht[:, tp * 2:tp * 2 + 2, :]
                                .rearrange("p t u -> p (t u)"), pst[:])
                        m0 = ci * 512
                        nc.sync.dma_start(
                            rhtm_d[b, m0:m0 + 512, :]
                            .rearrange("(t n) u -> n t u", t=4), rht[:])
                        # u
                        sgu = sm2.tile([128, 512], f32, tag="sg")
                        nc.scalar.activation(sgu[:], psw[1][ci][:], AF.Sigmoid,
                                             bias=wb[:, WGBC + 1:WGBC + 2])
                        nc.gpsimd.dma_start(ucm_d[:, b, c5], sgu[:])

            # ================= PHASE C: y1 = S @ rh =================
            with nc.named_scope("phaseC"):
                rh0 = st0.tile([128, KT, HW], f32r, tag="big")
                for kt in range(KT):
                    nc.sync.dma_start(
                        rh0[:, kt, :].rearrange("n (b u) -> n b u", b=BC),
                        rhtm_d[:, kt * 128:(kt + 1) * 128, :]
                        .rearrange("b n u -> n b u"))
                s_apply(rh0, HW, y1_d)

            # ============ PHASE D: cand + new_h + proj ============
            with nc.named_scope("phaseD"):
                for b in range(BC):
                    bc0 = b * 128
                    hcm = pcmB.tile([U, N], f32r, tag="hcm")
                    nc.sync.dma_start(hcm[:], hcm_d[:, b, :])
                    rhcm = pcmC.tile([U, N], f32r, tag="rhcm")
                    nc.sync.dma_start(rhcm[:], rhcm_d[:, b, :])
                    usb = pcmC.tile([U, N], f32, tag="usb")
                    nc.sync.dma_start(usb[:], ucm_d[:, b, :])
                    y1t = pcmA.tile([128, KT, 128], f32r, tag="x1t")
                    nc.sync.dma_start(
                        y1t[:], y1_d[:, bc0:bc0 + 128]
                        .rearrange("(k n) c -> n k c", n=128))
                    xi3 = sm2.tile([3, N], f32r, tag="xi3")
                    nc.sync.dma_start(xi3[:], xist_d[:, b * N:(b + 1) * N])
                    y1cm = pcmA.tile([U, N], f32r, tag="x1cm")
                    for np2 in range(KT // 2):
                        pst = ps2.tile([128, 256], f32, tag="tr")
                        for j in range(2):
                            nc.tensor.transpose(
                                pst[:, j * 128:(j + 1) * 128],
                                y1t[:, np2 * 2 + j, :].bitcast(f32), ident[:])
                        nc.scalar.copy(y1cm[:, np2 * 256:(np2 + 1) * 256], pst[:])
                    y2cm = pcmA.tile([U, N], f32r, tag="x2cm")
                    psx = [psAcc.tile([128, 512], f32, tag="acc",
                                      name=f"dx{b}_{ci}") for ci in range(2)]
                    for nt in range(KT):
                        for ci in range(2):
                            nc.tensor.matmul(
                                psx[ci][:], y1t[:, nt, :],
                                s_sb[:, nt, ci * 512:(ci + 1) * 512],
                                start=(nt == 0), stop=(nt == KT - 1))
                    for ci in range(2):
                        c5 = slice(ci * 512, (ci + 1) * 512)
                        nc.vector.scalar_tensor_tensor(
                            y2cm[:, c5], psx[ci][:], 2.0, rhcm[:, c5],
                            op0=ALU.mult, op1=ALU.subtract)
                    psw = [psAcc.tile([128, 512], f32, tag="acc",
                                      name=f"dw{b}_{ci}") for ci in range(2)]
                    rhs_k = [rhcm, y1cm, y2cm]
                    for k in range(3):
                        for ci in range(2):
                            c5 = slice(ci * 512, (ci + 1) * 512)
                            nc.tensor.matmul(psw[ci][:], wch_k(k),
                                             rhs_k[k][:, c5],
                                             start=(k == 0), stop=False)
                    for ci in range(2):
                        c5 = slice(ci * 512, (ci + 1) * 512)
                        nc.tensor.matmul(psw[ci][:], wb[0:3, WCXI:WCXI + 128],
                                         xi3[:, c5], start=False, stop=True)
                    for ci in range(2):
                        c5 = slice(ci * 512, (ci + 1) * 512)
                        t0 = b * N + ci * 512
                        csb = sm2.tile([128, 512], f32, tag="sg")
                        nc.scalar.activation(csb[:], psw[ci][:], AF.Tanh,
                                             bias=wb[:, WCBC:WCBC + 1])
                        # new_h = c + u*(h - c)
                        t1 = sm1.tile([128, 512], f32, tag="rhc")
                        nc.vector.tensor_sub(t1[:], hcm[:, c5], csb[:])
                        t2 = sm2.tile([128, 512], f32, tag="sg")
                        nc.vector.tensor_mul(t2[:], usb[:, c5], t1[:])
                        nh = sm1.tile([128, 512], f32r, tag="nh")
                        nc.vector.tensor_add(nh[:], csb[:], t2[:])
                        nc.gpsimd.dma_start(nh_d[:, b, c5], nh[:])
                        psp = ps2.tile([1, 512], f32, tag="sml")
                        nc.tensor.matmul(psp[:], wb[:, WPW0:WPW0 + 1], nh[:],
                                         start=True, stop=True)
                        po = sm2.tile([1, 512], f32, tag="po")
                        nc.scalar.activation(po[:], psp[:], AF.Identity,
                                             bias=wb[0:1, WPB:WPB + 1])
                        nc.gpsimd.dma_start(proj_d[:, t0:t0 + 512], po[:])

    nc.compile()
    return nc


def _prep(inputs, hidden_state, support, gate_w, gate_b, cand_w, cand_b,
          proj_w, proj_b):
    """Host-side sharding + layout prep. Returns list of per-core in_maps."""
    f = np.float32
    S_T = np.ascontiguousarray(np.asarray(support, f).T)
    h3 = np.asarray(hidden_state, f)[0].reshape(B, N, U)
    xi = np.asarray(inputs, f).reshape(B, N)
    gw = np.asarray(gate_w, f)
    cw = np.asarray(cand_w, f)
    # W rows: flat col index c*3+k ; c=0 is xi, c=1+j is hidden channel j
    hid_rows = 3 * (1 + np.arange(U))
    shared = dict(
        s_t=S_T,
        wgh=np.ascontiguousarray(np.stack([gw[hid_rows + k] for k in range(3)])),
        wgxi=np.ascontiguousarray(gw[0:3]),
        gb=np.asarray(gate_b, f).reshape(1, 2 * U),
        wch=np.ascontiguousarray(np.stack([cw[hid_rows + k] for k in range(3)])),
        wcxi=np.ascontiguousarray(cw[0:3]),
        cb=np.asarray(cand_b, f).reshape(1, U),
        pw=np.asarray(proj_w, f).reshape(U, 1),
        pb=np.asarray(proj_b, f).reshape(1, 1),
    )
    maps = []
    for c in range(NCORES):
        b0 = c * BC
        h_tm = np.ascontiguousarray(h3[b0:b0 + BC])               # [BC, N, U]
        h_cm = np.ascontiguousarray(h_tm.transpose(2, 0, 1))      # [U, BC, N]
        xib = np.ascontiguousarray(xi[b0:b0 + BC])                # [BC, N]
        maps.append(dict(
            shared,
            h_tm=h_tm, h_cm=h_cm,
            xi_nm=np.ascontiguousarray(xib.T),
            xi_bm=xib,
        ))
    return maps


def _gather(results):
    out = np.empty((B, N * OD), np.float32)
    new_h = np.empty((B, N * U), np.float32)
    for c in range(NCORES):
        b0 = c * BC
        r = results[c]
        out[b0:b0 + BC] = r["out_proj"].reshape(BC, N)
        new_h[b0:b0 + BC] = r["nh_cm"].transpose(1, 2, 0).reshape(BC, N * U)
    return out, new_h[None]


def kernel(**inputs):
    from concourse.bass_utils import run_bass_kernel_spmd

    if "nc" not in _cache:
        _cache["nc"] = _build()
    nc = _cache["nc"]
    maps = _prep(**inputs)
    res = run_bass_kernel_spmd(nc, maps, core_ids=list(range(NCORES)),
                               trace=False)
    _cache["last_res"] = res
    return _gather(res.results)


# revision 9
# speedup vs baseline: 1.3128x; 1.0092x over previous
"""DCRNN decoder cell (gnn_message_passing) on 8 Trainium2 cores.

Data-parallel over batch B=128 -> 16 batches/core; support matrix and
weights replicated; no collectives.

Per core (Bc=16):
  x0 = concat([xi, h]); x1 = S x0; x2 = 2 S x1 - x0     (Chebyshev)
  gate = sigmoid(sum_k Xk @ Wg_k + bg) -> r, u
  rh = r*h; y1 = S rh; y2 = 2 S y1 - rh
  c = tanh(sum_k Yk @ Wc_k + bc)
  new_h = u h + (1-u) c ; out = new_h @ pw + pb

Layouts: node-major [n, (b,u)] for Chebyshev; channel-major [u, (b,n)]
for the W-stages (out_T[o,tok] = sum_c Wk[c,o] Xk_cm[c,tok], bias and
the xi channel folded in as K=1 / K=3 matmuls).  x2_cm is computed
directly as (2 S x1 - x0)_cm via lhsT=x1_nm strips, rhs=S^T: the matmul
performs the S-apply and the cm-transpose at once.  x1_cm comes from PE
transposes of x1_nm strips.  All matmuls in float32r (full PE rate).
"""
import numpy as np

import concourse.bacc as bacc
import concourse.tile as tile
from concourse import mybir
from concourse.masks import make_identity

f32 = mybir.dt.float32
f32r = mybir.dt.float32r
AF = mybir.ActivationFunctionType
ALU = mybir.AluOpType

B, N, U, OD = 128, 1024, 128, 1
NCORES = 8
BC = B // NCORES            # 16 batches per core
KT = N // 128               # 8 node tiles
TOK = BC * N                # 16384 tokens per core
HW = BC * U                 # 2048 hidden cols
XW = HW + BC                # 2064 = hidden + xi cols

# packed small-weights tile column offsets (f32r [128, 1792])
WGH0 = 0            # [128, 3*256] gate hidden weights, c-major
WCH0 = 768          # [128, 3*128] cand hidden weights
WPW0 = WCH0 + 384   # [128, 1] proj weight
WROW = WPW0 + 1     # row-tensors region (partitions 0..2):
#   wgxi [3, 256] @ WROW, wcxi [3, 128] @ +256, gb [1, 256] @ +384 (p0),
#   cb [1,128] @ +640 (p0), pb [1,1] @ +768 (p0), ones [1,512] @ +769 (p0)
WGXI = WROW
WCXI = WROW + 256
WGBC = WCXI + 128   # [128, 2] gate bias as columns (r, u)
WCBC = WGBC + 2     # [128, 1] cand bias column
WPB = WCBC + 1      # [1, 1] proj bias
WBIGW = WPB + 1

_cache = {}


def _build():
    nc = bacc.Bacc("TRN2", target_bir_lowering=False, debug=False)

    s_d = nc.dram_tensor("s_t", [N, N], f32r, kind="ExternalInput").ap()
    htm_d = nc.dram_tensor("h_tm", [BC, N, U], f32r, kind="ExternalInput").ap()
    hcm_d = nc.dram_tensor("h_cm", [U, BC, N], f32r, kind="ExternalInput").ap()
    xinm_d = nc.dram_tensor("xi_nm", [N, BC], f32r, kind="ExternalInput").ap()
    xibm_d = nc.dram_tensor("xi_bm", [BC, N], f32r, kind="ExternalInput").ap()
    wgh_d = nc.dram_tensor("wgh", [3, U, 2 * U], f32r, kind="ExternalInput").ap()
    wgxi_d = nc.dram_tensor("wgxi", [3, 2 * U], f32r, kind="ExternalInput").ap()
    gb_d = nc.dram_tensor("gb", [1, 2 * U], f32r, kind="ExternalInput").ap()
    wch_d = nc.dram_tensor("wch", [3, U, U], f32r, kind="ExternalInput").ap()
    wcxi_d = nc.dram_tensor("wcxi", [3, U], f32r, kind="ExternalInput").ap()
    cb_d = nc.dram_tensor("cb", [1, U], f32r, kind="ExternalInput").ap()
    pw_d = nc.dram_tensor("pw", [U, 1], f32r, kind="ExternalInput").ap()
    pb_d = nc.dram_tensor("pb", [1, 1], f32r, kind="ExternalInput").ap()

    proj_d = nc.dram_tensor("out_proj", [1, TOK], f32, kind="ExternalOutput").ap()
    nh_d = nc.dram_tensor("nh_cm", [U, BC, N], f32r, kind="ExternalOutput").ap()

    x1_d = nc.dram_tensor("x1_nm", [N, XW], f32r, kind="Internal").ap()
    y1_d = nc.dram_tensor("y1_nm", [N, HW], f32r, kind="Internal").ap()
    rhtm_d = nc.dram_tensor("rh_tm", [BC, N, U], f32r, kind="Internal").ap()
    rhcm_d = nc.dram_tensor("rh_cm", [U, BC, N], f32r, kind="Internal").ap()
    ucm_d = nc.dram_tensor("u_cm", [U, BC, N], f32, kind="Internal").ap()
    xist_d = nc.dram_tensor("xi_stack", [3, TOK], f32r, kind="Internal").ap()

    with tile.TileContext(nc) as tc:
        with (
            tc.tile_pool(name="consts", bufs=1) as consts,
            tc.tile_pool(name="stage1", bufs=1) as st1,
            tc.tile_pool(name="stage0", bufs=1) as st0,
            tc.tile_pool(name="pcmA", bufs=2) as pcmA,
            tc.tile_pool(name="pcmB", bufs=2) as pcmB,
            tc.tile_pool(name="pcmC", bufs=2) as pcmC,
            tc.tile_pool(name="sm2", bufs=2) as sm2,
            tc.tile_pool(name="sm1", bufs=2) as sm1,
            tc.tile_pool(name="ps", bufs=4, space="PSUM") as psAcc,
            tc.tile_pool(name="ps2", bufs=2, space="PSUM") as ps2,
        ):
            # ---------------- constants ----------------
            ident = consts.tile([128, 128], f32)
            make_identity(nc, ident)
            s_sb = consts.tile([128, KT, N], f32r)
            for kt in range(KT):
                nc.sync.dma_start(s_sb[:, kt, :], s_d[kt * 128:(kt + 1) * 128, :])
            wb = consts.tile([128, WBIGW], f32r)
            nc.sync.dma_start(
                wb[:, WGH0:WGH0 + 768].rearrange("c (k o) -> c k o", k=3),
                wgh_d[:, :, :].rearrange("k c o -> c k o"))
            nc.sync.dma_start(
                wb[:, WCH0:WCH0 + 384].rearrange("c (k o) -> c k o", k=3),
                wch_d[:, :, :].rearrange("k c o -> c k o"))
            nc.sync.dma_start(wb[:, WPW0:WPW0 + 1], pw_d[:, :])
            nc.sync.dma_start(wb[0:3, WGXI:WGXI + 256], wgxi_d[:, :])
            nc.sync.dma_start(wb[0:3, WCXI:WCXI + 128], wcxi_d[:, :])
            nc.sync.dma_start(wb[:, WGBC:WGBC + 2],
                              gb_d[:, :].rearrange("one (t c) -> c (one t)", t=2))
            nc.sync.dma_start(wb[:, WCBC:WCBC + 1],
                              cb_d[:, :].rearrange("one c -> c one"))
            nc.sync.dma_start(wb[0:1, WPB:WPB + 1], pb_d[:, :])

            def wgh_k(k, o0):
                return wb[:, WGH0 + k * 256 + o0:WGH0 + k * 256 + o0 + 128]

            def wch_k(k):
                return wb[:, WCH0 + k * 128:WCH0 + (k + 1) * 128]

            # -------- helper: node-major S-apply, resident source --------
            def s_apply(src_res, width, dst_dram):
                """dst_dram[mt-block, :width] = S @ src_res ([128, KT, width]).
                mt/kt-outer so each stationary S-tile serves all chunks."""
                nch = width // 512
                tail = width - nch * 512
                for mt in range(KT):
                    pss = [psAcc.tile([128, 512], f32, tag="acc",
                                      name=f"apl{mt}_{ci}")
                           for ci in range(nch)]
                    pxi = (ps2.tile([128, 128], f32, tag="tr",
                                    name=f"apx{mt}")
                           if tail else None)
                    for kt in range(KT):
                        for ci in range(nch):
                            nc.tensor.matmul(
                                pss[ci][:], s_sb[:, kt, mt * 128:(mt + 1) * 128],
                                src_res[:, kt, ci * 512:(ci + 1) * 512],
                                start=(kt == 0), stop=(kt == KT - 1))
                        if tail:
                            nc.tensor.matmul(
                                pxi[:, 0:tail],
                                s_sb[:, kt, mt * 128:(mt + 1) * 128],
                                src_res[:, kt, nch * 512:width],
                                start=(kt == 0), stop=(kt == KT - 1))
                    for ci in range(nch):
                        so = sm2.tile([128, 512], f32r, tag="sg",
                                      name=f"so{mt}_{ci}")
                        nc.vector.tensor_copy(so[:], pss[ci][:])
                        nc.sync.dma_start(
                            dst_dram[mt * 128:(mt + 1) * 128,
                                     ci * 512:(ci + 1) * 512], so[:])
                    if tail:
                        sot = sm2.tile([128, tail], f32r, tag="po",
                                       name=f"sot{mt}")
                        nc.vector.tensor_copy(sot[:], pxi[:, 0:tail])
                        nc.sync.dma_start(
                            dst_dram[mt * 128:(mt + 1) * 128, nch * 512:width],
                            sot[:])

            # ================= PHASE A: x1 = S @ x0 =================
            with nc.named_scope("phaseA"):
                x0 = st0.tile([128, KT, XW], f32r, tag="big")
                for kt in range(KT):
                    nc.sync.dma_start(
                        x0[:, kt, 0:HW].rearrange("n (b u) -> n b u", b=BC),
                        htm_d[:, kt * 128:(kt + 1) * 128, :]
                        .rearrange("b n u -> n b u"))
                    nc.sync.dma_start(x0[:, kt, HW:XW],
                                      xinm_d[kt * 128:(kt + 1) * 128, :])
                s_apply(x0, XW, x1_d)

                # xi chebyshev rows -> DRAM xi_stack
                with tc.tile_pool(name="xip", bufs=1) as xip:
                    xibm = xip.tile([BC, N], f32r)
                    nc.sync.dma_start(xibm[:], xibm_d[:, :])
                    nc.sync.dma_start(xist_d[0:1, :], xibm[:, :])
                    x1xit = xip.tile([128, KT, BC], f32r)
                    nc.sync.dma_start(
                        x1xit[:], x1_d[:, HW:XW].rearrange("(k n) c -> n k c",
                                                           n=128))
                    x1xi = xip.tile([BC, N], f32r)
                    x2xi = xip.tile([BC, N], f32r)
                    for ch in range(2):
                        c5 = slice(ch * 512, (ch + 1) * 512)
                        ps = ps2.tile([BC, 512], f32, tag="sml")
                        for kt in range(KT):
                            nc.tensor.matmul(ps[:], x0[:, kt, HW:XW],
                                             s_sb[:, kt, c5],
                                             start=(kt == 0), stop=(kt == KT - 1))
                        nc.vector.tensor_copy(x1xi[:, c5], ps[:])
                        psb = ps2.tile([BC, 512], f32, tag="sml")
                        for nt in range(KT):
                            nc.tensor.matmul(psb[:], x1xit[:, nt, :],
                                             s_sb[:, nt, c5],
                                             start=(nt == 0), stop=(nt == KT - 1))
                        nc.vector.scalar_tensor_tensor(
                            x2xi[:, c5], psb[:], 2.0, xibm[:, c5],
                            op0=ALU.mult, op1=ALU.subtract)
                    nc.sync.dma_start(xist_d[1:2, :], x1xi[:, :])
                    nc.sync.dma_start(xist_d[2:3, :], x2xi[:, :])

            # ============ PHASE B: per-batch gate pipeline ============
            with nc.named_scope("phaseB"):
                for b in range(BC):
                    bc0 = b * 128
                    hcm = pcmB.tile([U, N], f32r, tag="hcm")
                    nc.sync.dma_start(hcm[:], hcm_d[:, b, :])
                    x1t = pcmA.tile([128, KT, 128], f32r, tag="x1t")
                    nc.sync.dma_start(
                        x1t[:], x1_d[:, bc0:bc0 + 128]
                        .rearrange("(k n) c -> n k c", n=128))
                    xi3 = sm2.tile([3, N], f32r, tag="xi3")
                    nc.sync.dma_start(xi3[:], xist_d[:, b * N:(b + 1) * N])
                    x1cm = pcmA.tile([U, N], f32r, tag="x1cm")
                    for np2 in range(KT // 2):
                        pst = ps2.tile([128, 256], f32, tag="tr")
                        for j in range(2):
                            nc.tensor.transpose(
                                pst[:, j * 128:(j + 1) * 128],
                                x1t[:, np2 * 2 + j, :].bitcast(f32), ident[:])
                        nc.scalar.copy(x1cm[:, np2 * 256:(np2 + 1) * 256], pst[:])
                    x2cm = pcmA.tile([U, N], f32r, tag="x2cm")
                    psx = [psAcc.tile([128, 512], f32, tag="acc",
                                      name=f"bx{b}_{ci}") for ci in range(2)]
                    for nt in range(KT):
                        for ci in range(2):
                            nc.tensor.matmul(
                                psx[ci][:], x1t[:, nt, :],
                                s_sb[:, nt, ci * 512:(ci + 1) * 512],
                                start=(nt == 0), stop=(nt == KT - 1))
                    for ci in range(2):
                        c5 = slice(ci * 512, (ci + 1) * 512)
                        nc.vector.scalar_tensor_tensor(
                            x2cm[:, c5], psx[ci][:], 2.0, hcm[:, c5],
                            op0=ALU.mult, op1=ALU.subtract)
                    # gate W-stage: 4 psums (2 ot x 2 ch), k-outer
                    psw = [[psAcc.tile([128, 512], f32, tag="acc",
                                       name=f"bw{b}_{ot}_{ci}")
                            for ci in range(2)] for ot in range(2)]
                    rhs_k = [hcm, x1cm, x2cm]
                    for ot in range(2):
                        o0 = ot * 128
                        for k in range(3):
                            for ci in range(2):
                                c5 = slice(ci * 512, (ci + 1) * 512)
                                nc.tensor.matmul(psw[ot][ci][:], wgh_k(k, o0),
                                                 rhs_k[k][:, c5],
                                                 start=(k == 0), stop=False)
                        for ci in range(2):
                            c5 = slice(ci * 512, (ci + 1) * 512)
                            nc.tensor.matmul(psw[ot][ci][:],
                                             wb[0:3, WGXI + o0:WGXI + o0 + 128],
                                             xi3[:, c5], start=False, stop=True)
                    for ci in range(2):
                        c5 = slice(ci * 512, (ci + 1) * 512)
                        # r
                        sgr = sm2.tile([128, 512], f32, tag="sg")
                        nc.scalar.activation(sgr[:], psw[0][ci][:], AF.Sigmoid,
                                             bias=wb[:, WGBC:WGBC + 1])
                        rhc = sm1.tile([128, 512], f32r, tag="rhc")
                        nc.vector.tensor_mul(rhc[:], sgr[:], hcm[:, c5])
                        nc.gpsimd.dma_start(rhcm_d[:, b, c5], rhc[:])
                        rht = sm2.tile([128, 4, U], f32r, tag="rht")
                        for tp in range(2):
                            pst = ps2.tile([128, 256], f32, tag="tr")
                            for j in range(2):
                                nc.tensor.transpose(
                                    pst[:, j * 128:(j + 1) * 128],
                                    rhc[:, (tp * 2 + j) * 128:
                                        (tp * 2 + j + 1) * 128]
                                    .bitcast(f32), ident[:])
                            nc.vector.tensor_copy(
                                rht[:, tp * 2:tp * 2 + 2, :]
                                .rearrange("p t u -> p (t u)"), pst[:])
                        m0 = ci * 512
                        nc.sync.dma_start(
                            rhtm_d[b, m0:m0 + 512, :]
                            .rearrange("(t n) u -> n t u", t=4), rht[:])
                        # u
                        sgu = sm2.tile([128, 512], f32, tag="sg")
                        nc.scalar.activation(sgu[:], psw[1][ci][:], AF.Sigmoid,
                                             bias=wb[:, WGBC + 1:WGBC + 2])
                        nc.gpsimd.dma_start(ucm_d[:, b, c5], sgu[:])

            # ================= PHASE C: y1 = S @ rh =================
            with nc.named_scope("phaseC"):
                rh0 = st0.tile([128, KT, HW], f32r, tag="big")
                for kt in range(KT):
                    nc.sync.dma_start(
                        rh0[:, kt, :].rearrange("n (b u) -> n b u", b=BC),
                        rhtm_d[:, kt * 128:(kt + 1) * 128, :]
                        .rearrange("b n u -> n b u"))
                s_apply(rh0, HW, y1_d)

            # ============ PHASE D: cand + new_h + proj ============
            with nc.named_scope("phaseD"):
                for b in range(BC):
                    bc0 = b * 128
                    hcm = pcmB.tile([U, N], f32r, tag="hcm")
                    nc.sync.dma_start(hcm[:], hcm_d[:, b, :])
                    rhcm = pcmC.tile([U, N], f32r, tag="rhcm")
                    nc.sync.dma_start(rhcm[:], rhcm_d[:, b, :])
                    usb = pcmC.tile([U, N], f32, tag="usb")
                    nc.sync.dma_start(usb[:], ucm_d[:, b, :])
                    y1t = pcmA.tile([128, KT, 128], f32r, tag="x1t")
                    nc.sync.dma_start(
                        y1t[:], y1_d[:, bc0:bc0 + 128]
                        .rearrange("(k n) c -> n k c", n=128))
                    xi3 = sm2.tile([3, N], f32r, tag="xi3")
                    nc.sync.dma_start(xi3[:], xist_d[:, b * N:(b + 1) * N])
                    y1cm = pcmA.tile([U, N], f32r, tag="x1cm")
                    for np2 in range(KT // 2):
                        pst = ps2.tile([128, 256], f32, tag="tr")
                        for j in range(2):
                            nc.tensor.transpose(
                                pst[:, j * 128:(j + 1) * 128],
                                y1t[:, np2 * 2 + j, :].bitcast(f32), ident[:])
                        nc.scalar.copy(y1cm[:, np2 * 256:(np2 + 1) * 256], pst[:])
                    y2cm = pcmA.tile([U, N], f32r, tag="x2cm")
                    psx = [psAcc.tile([128, 512], f32, tag="acc",
                                      name=f"dx{b}_{ci}") for ci in range(2)]
                    for nt in range(KT):
                        for ci in range(2):
                            nc.tensor.matmul(
                                psx[ci][:], y1t[:, nt, :],
                                s_sb[:, nt, ci * 512:(ci + 1) * 512],
                                start=(nt == 0), stop=(nt == KT - 1))
                    for ci in range(2):
                        c5 = slice(ci * 512, (ci + 1) * 512)
                        nc.vector.scalar_tensor_tensor(
                            y2cm[:, c5], psx[ci][:], 2.0, rhcm[:, c5],
                            op0=ALU.mult, op1=ALU.subtract)
                    psw = [psAcc.tile([128, 512], f32, tag="acc",
                                      name=f"dw{b}_{ci}") for ci in range(2)]
                    rhs_k = [rhcm, y1cm, y2cm]
                    for k in range(3):
                        for ci in range(2):
                            c5 = slice(ci * 512, (ci + 1) * 512)
                            nc.tensor.matmul(psw[ci][:], wch_k(k),
                                             rhs_k[k][:, c5],
                                             start=(k == 0), stop=False)
                    for ci in range(2):
                        c5 = slice(ci * 512, (ci + 1) * 512)
                        nc.tensor.matmul(psw[ci][:], wb[0:3, WCXI:WCXI + 128],
                                         xi3[:, c5], start=False, stop=True)
                    for ci in range(2):
                        c5 = slice(ci * 512, (ci + 1) * 512)
                        t0 = b * N + ci * 512
                        # z1 = u*h computed before tanh lands
                        z1 = sm1.tile([128, 512], f32, tag="rhc")
                        nc.vector.tensor_mul(z1[:], usb[:, c5], hcm[:, c5])
                        csb = sm2.tile([128, 512], f32, tag="sg")
                        nc.scalar.activation(csb[:], psw[ci][:], AF.Tanh,
                                             bias=wb[:, WCBC:WCBC + 1])
                        # new_h = z1 - (u-1)*c
                        t2 = sm2.tile([128, 512], f32, tag="sg")
                        nc.vector.scalar_tensor_tensor(
                            t2[:], usb[:, c5], 1.0, csb[:],
                            op0=ALU.subtract, op1=ALU.mult)
                        nh = sm1.tile([128, 512], f32r, tag="nh")
                        nc.vector.tensor_sub(nh[:], z1[:], t2[:])
                        nc.gpsimd.dma_start(nh_d[:, b, c5], nh[:])
                        psp = ps2.tile([1, 512], f32, tag="sml")
                        nc.tensor.matmul(psp[:], wb[:, WPW0:WPW0 + 1], nh[:],
                                         start=True, stop=True)
                        po = sm2.tile([1, 512], f32, tag="po")
                        nc.scalar.activation(po[:], psp[:], AF.Identity,
                                             bias=wb[0:1, WPB:WPB + 1])
                        nc.gpsimd.dma_start(proj_d[:, t0:t0 + 512], po[:])

    nc.compile()
    return nc


def _prep(inputs, hidden_state, support, gate_w, gate_b, cand_w, cand_b,
          proj_w, proj_b):
    """Host-side sharding + layout prep. Returns list of per-core in_maps."""
    f = np.float32
    S_T = np.ascontiguousarray(np.asarray(support, f).T)
    h3 = np.asarray(hidden_state, f)[0].reshape(B, N, U)
    xi = np.asarray(inputs, f).reshape(B, N)
    gw = np.asarray(gate_w, f)
    cw = np.asarray(cand_w, f)
    # W rows: flat col index c*3+k ; c=0 is xi, c=1+j is hidden channel j
    hid_rows = 3 * (1 + np.arange(U))
    shared = dict(
        s_t=S_T,
        wgh=np.ascontiguousarray(np.stack([gw[hid_rows + k] for k in range(3)])),
        wgxi=np.ascontiguousarray(gw[0:3]),
        gb=np.asarray(gate_b, f).reshape(1, 2 * U),
        wch=np.ascontiguousarray(np.stack([cw[hid_rows + k] for k in range(3)])),
        wcxi=np.ascontiguousarray(cw[0:3]),
        cb=np.asarray(cand_b, f).reshape(1, U),
        pw=np.asarray(proj_w, f).reshape(U, 1),
        pb=np.asarray(proj_b, f).reshape(1, 1),
    )
    maps = []
    for c in range(NCORES):
        b0 = c * BC
        h_tm = np.ascontiguousarray(h3[b0:b0 + BC])               # [BC, N, U]
        h_cm = np.ascontiguousarray(h_tm.transpose(2, 0, 1))      # [U, BC, N]
        xib = np.ascontiguousarray(xi[b0:b0 + BC])                # [BC, N]
        maps.append(dict(
            shared,
            h_tm=h_tm, h_cm=h_cm,
            xi_nm=np.ascontiguousarray(xib.T),
            xi_bm=xib,
        ))
    return maps


def _gather(results):
    out = np.empty((B, N * OD), np.float32)
    new_h = np.empty((B, N * U), np.float32)
    for c in range(NCORES):
        b0 = c * BC
        r = results[c]
        out[b0:b0 + BC] = r["out_proj"].reshape(BC, N)
        new_h[b0:b0 + BC] = r["nh_cm"].transpose(1, 2, 0).reshape(BC, N * U)
    return out, new_h[None]


def kernel(**inputs):
    from concourse.bass_utils import run_bass_kernel_spmd

    if "nc" not in _cache:
        _cache["nc"] = _build()
    nc = _cache["nc"]
    maps = _prep(**inputs)
    res = run_bass_kernel_spmd(nc, maps, core_ids=list(range(NCORES)),
                               trace=False)
    _cache["last_res"] = res
    return _gather(res.results)


# revision 10
# speedup vs baseline: 1.3133x; 1.0004x over previous
"""DCRNN decoder cell (gnn_message_passing) on 8 Trainium2 cores.

Data-parallel over batch B=128 -> 16 batches/core; support matrix and
weights replicated; no collectives.

Per core (Bc=16):
  x0 = concat([xi, h]); x1 = S x0; x2 = 2 S x1 - x0     (Chebyshev)
  gate = sigmoid(sum_k Xk @ Wg_k + bg) -> r, u
  rh = r*h; y1 = S rh; y2 = 2 S y1 - rh
  c = tanh(sum_k Yk @ Wc_k + bc)
  new_h = u h + (1-u) c ; out = new_h @ pw + pb

Layouts: node-major [n, (b,u)] for Chebyshev; channel-major [u, (b,n)]
for the W-stages (out_T[o,tok] = sum_c Wk[c,o] Xk_cm[c,tok], bias and
the xi channel folded in as K=1 / K=3 matmuls).  x2_cm is computed
directly as (2 S x1 - x0)_cm via lhsT=x1_nm strips, rhs=S^T: the matmul
performs the S-apply and the cm-transpose at once.  x1_cm comes from PE
transposes of x1_nm strips.  All matmuls in float32r (full PE rate).
"""
import numpy as np

import concourse.bacc as bacc
import concourse.tile as tile
from concourse import mybir
from concourse.masks import make_identity

f32 = mybir.dt.float32
f32r = mybir.dt.float32r
AF = mybir.ActivationFunctionType
ALU = mybir.AluOpType

B, N, U, OD = 128, 1024, 128, 1
NCORES = 8
BC = B // NCORES            # 16 batches per core
KT = N // 128               # 8 node tiles
TOK = BC * N                # 16384 tokens per core
HW = BC * U                 # 2048 hidden cols
XW = HW + BC                # 2064 = hidden + xi cols

# packed small-weights tile column offsets (f32r [128, 1792])
WGH0 = 0            # [128, 3*256] gate hidden weights, c-major
WCH0 = 768          # [128, 3*128] cand hidden weights
WPW0 = WCH0 + 384   # [128, 1] proj weight
WROW = WPW0 + 1     # row-tensors region (partitions 0..2):
#   wgxi [3, 256] @ WROW, wcxi [3, 128] @ +256, gb [1, 256] @ +384 (p0),
#   cb [1,128] @ +640 (p0), pb [1,1] @ +768 (p0), ones [1,512] @ +769 (p0)
WGXI = WROW
WCXI = WROW + 256
WGBC = WCXI + 128   # [128, 2] gate bias as columns (r, u)
WCBC = WGBC + 2     # [128, 1] cand bias column
WPB = WCBC + 1      # [1, 1] proj bias
WBIGW = WPB + 1

_cache = {}


def _build():
    nc = bacc.Bacc("TRN2", target_bir_lowering=False, debug=False)

    s_d = nc.dram_tensor("s_t", [N, N], f32r, kind="ExternalInput").ap()
    hnm_d = nc.dram_tensor("h_nm", [N, BC * U], f32r, kind="ExternalInput").ap()
    hcm_d = nc.dram_tensor("h_cm", [U, BC, N], f32r, kind="ExternalInput").ap()
    xinm_d = nc.dram_tensor("xi_nm", [N, BC], f32r, kind="ExternalInput").ap()
    xibm_d = nc.dram_tensor("xi_bm", [BC, N], f32r, kind="ExternalInput").ap()
    wgh_d = nc.dram_tensor("wgh", [3, U, 2 * U], f32r, kind="ExternalInput").ap()
    wgxi_d = nc.dram_tensor("wgxi", [3, 2 * U], f32r, kind="ExternalInput").ap()
    gb_d = nc.dram_tensor("gb", [1, 2 * U], f32r, kind="ExternalInput").ap()
    wch_d = nc.dram_tensor("wch", [3, U, U], f32r, kind="ExternalInput").ap()
    wcxi_d = nc.dram_tensor("wcxi", [3, U], f32r, kind="ExternalInput").ap()
    cb_d = nc.dram_tensor("cb", [1, U], f32r, kind="ExternalInput").ap()
    pw_d = nc.dram_tensor("pw", [U, 1], f32r, kind="ExternalInput").ap()
    pb_d = nc.dram_tensor("pb", [1, 1], f32r, kind="ExternalInput").ap()

    proj_d = nc.dram_tensor("out_proj", [1, TOK], f32, kind="ExternalOutput").ap()
    nh_d = nc.dram_tensor("nh_cm", [U, BC, N], f32r, kind="ExternalOutput").ap()

    x1_d = nc.dram_tensor("x1_nm", [N, XW], f32r, kind="Internal").ap()
    y1_d = nc.dram_tensor("y1_nm", [N, HW], f32r, kind="Internal").ap()
    rhnm_d = nc.dram_tensor("rh_nm", [N, BC, U], f32r, kind="Internal").ap()
    rhcm_d = nc.dram_tensor("rh_cm", [U, BC, N], f32r, kind="Internal").ap()
    ucm_d = nc.dram_tensor("u_cm", [U, BC, N], f32, kind="Internal").ap()
    xist_d = nc.dram_tensor("xi_stack", [3, TOK], f32r, kind="Internal").ap()

    with tile.TileContext(nc) as tc:
        with (
            tc.tile_pool(name="consts", bufs=1) as consts,
            tc.tile_pool(name="stage1", bufs=1) as st1,
            tc.tile_pool(name="stage0", bufs=1) as st0,
            tc.tile_pool(name="pcmA", bufs=2) as pcmA,
            tc.tile_pool(name="pcmB", bufs=2) as pcmB,
            tc.tile_pool(name="pcmC", bufs=2) as pcmC,
            tc.tile_pool(name="sm2", bufs=2) as sm2,
            tc.tile_pool(name="sm1", bufs=2) as sm1,
            tc.tile_pool(name="ps", bufs=4, space="PSUM") as psAcc,
            tc.tile_pool(name="ps2", bufs=2, space="PSUM") as ps2,
        ):
            # ---------------- constants ----------------
            ident = consts.tile([128, 128], f32)
            make_identity(nc, ident)
            s_sb = consts.tile([128, KT, N], f32r)
            for kt in range(KT):
                nc.sync.dma_start(s_sb[:, kt, :], s_d[kt * 128:(kt + 1) * 128, :])
            wb = consts.tile([128, WBIGW], f32r)
            nc.sync.dma_start(
                wb[:, WGH0:WGH0 + 768].rearrange("c (k o) -> c k o", k=3),
                wgh_d[:, :, :].rearrange("k c o -> c k o"))
            nc.sync.dma_start(
                wb[:, WCH0:WCH0 + 384].rearrange("c (k o) -> c k o", k=3),
                wch_d[:, :, :].rearrange("k c o -> c k o"))
            nc.sync.dma_start(wb[:, WPW0:WPW0 + 1], pw_d[:, :])
            nc.sync.dma_start(wb[0:3, WGXI:WGXI + 256], wgxi_d[:, :])
            nc.sync.dma_start(wb[0:3, WCXI:WCXI + 128], wcxi_d[:, :])
            nc.sync.dma_start(wb[:, WGBC:WGBC + 2],
                              gb_d[:, :].rearrange("one (t c) -> c (one t)", t=2))
            nc.sync.dma_start(wb[:, WCBC:WCBC + 1],
                              cb_d[:, :].rearrange("one c -> c one"))
            nc.sync.dma_start(wb[0:1, WPB:WPB + 1], pb_d[:, :])

            def wgh_k(k, o0):
                return wb[:, WGH0 + k * 256 + o0:WGH0 + k * 256 + o0 + 128]

            def wch_k(k):
                return wb[:, WCH0 + k * 128:WCH0 + (k + 1) * 128]

            # -------- helper: node-major S-apply, resident source --------
            def s_apply(src_res, width, dst_dram):
                """dst_dram[mt-block, :width] = S @ src_res ([128, KT, width]).
                mt/kt-outer so each stationary S-tile serves all chunks."""
                nch = width // 512
                tail = width - nch * 512
                for mt in range(KT):
                    pss = [psAcc.tile([128, 512], f32, tag="acc",
                                      name=f"apl{mt}_{ci}")
                           for ci in range(nch)]
                    pxi = (ps2.tile([128, 128], f32, tag="tr",
                                    name=f"apx{mt}")
                           if tail else None)
                    for kt in range(KT):
                        for ci in range(nch):
                            nc.tensor.matmul(
                                pss[ci][:], s_sb[:, kt, mt * 128:(mt + 1) * 128],
                                src_res[:, kt, ci * 512:(ci + 1) * 512],
                                start=(kt == 0), stop=(kt == KT - 1))
                        if tail:
                            nc.tensor.matmul(
                                pxi[:, 0:tail],
                                s_sb[:, kt, mt * 128:(mt + 1) * 128],
                                src_res[:, kt, nch * 512:width],
                                start=(kt == 0), stop=(kt == KT - 1))
                    for ci in range(nch):
                        so = sm2.tile([128, 512], f32r, tag="sg",
                                      name=f"so{mt}_{ci}")
                        nc.vector.tensor_copy(so[:], pss[ci][:])
                        nc.sync.dma_start(
                            dst_dram[mt * 128:(mt + 1) * 128,
                                     ci * 512:(ci + 1) * 512], so[:])
                    if tail:
                        sot = sm2.tile([128, tail], f32r, tag="po",
                                       name=f"sot{mt}")
                        nc.vector.tensor_copy(sot[:], pxi[:, 0:tail])
                        nc.sync.dma_start(
                            dst_dram[mt * 128:(mt + 1) * 128, nch * 512:width],
                            sot[:])

            # ================= PHASE A: x1 = S @ x0 =================
            with nc.named_scope("phaseA"):
                x0 = st0.tile([128, KT, XW], f32r, tag="big")
                for kt in range(KT):
                    eng = nc.sync if kt % 2 == 0 else nc.scalar
                    eng.dma_start(x0[:, kt, 0:HW],
                                  hnm_d[kt * 128:(kt + 1) * 128, :])
                    eng.dma_start(x0[:, kt, HW:XW],
                                  xinm_d[kt * 128:(kt + 1) * 128, :])
                s_apply(x0, XW, x1_d)

                # xi chebyshev rows -> DRAM xi_stack
                with tc.tile_pool(name="xip", bufs=1) as xip:
                    xibm = xip.tile([BC, N], f32r)
                    nc.sync.dma_start(xibm[:], xibm_d[:, :])
                    nc.sync.dma_start(xist_d[0:1, :], xibm[:, :])
                    x1xit = xip.tile([128, KT, BC], f32r)
                    nc.sync.dma_start(
                        x1xit[:], x1_d[:, HW:XW].rearrange("(k n) c -> n k c",
                                                           n=128))
                    x1xi = xip.tile([BC, N], f32r)
                    x2xi = xip.tile([BC, N], f32r)
                    for ch in range(2):
                        c5 = slice(ch * 512, (ch + 1) * 512)
                        ps = ps2.tile([BC, 512], f32, tag="sml")
                        for kt in range(KT):
                            nc.tensor.matmul(ps[:], x0[:, kt, HW:XW],
                                             s_sb[:, kt, c5],
                                             start=(kt == 0), stop=(kt == KT - 1))
                        nc.vector.tensor_copy(x1xi[:, c5], ps[:])
                        psb = ps2.tile([BC, 512], f32, tag="sml")
                        for nt in range(KT):
                            nc.tensor.matmul(psb[:], x1xit[:, nt, :],
                                             s_sb[:, nt, c5],
                                             start=(nt == 0), stop=(nt == KT - 1))
                        nc.vector.scalar_tensor_tensor(
                            x2xi[:, c5], psb[:], 2.0, xibm[:, c5],
                            op0=ALU.mult, op1=ALU.subtract)
                    nc.sync.dma_start(xist_d[1:2, :], x1xi[:, :])
                    nc.sync.dma_start(xist_d[2:3, :], x2xi[:, :])

            # ============ PHASE B: per-batch gate pipeline ============
            with nc.named_scope("phaseB"):
                for b in range(BC):
                    bc0 = b * 128
                    hcm = pcmB.tile([U, N], f32r, tag="hcm")
                    nc.sync.dma_start(hcm[:], hcm_d[:, b, :])
                    x1t = pcmA.tile([128, KT, 128], f32r, tag="x1t")
                    nc.sync.dma_start(
                        x1t[:], x1_d[:, bc0:bc0 + 128]
                        .rearrange("(k n) c -> n k c", n=128))
                    xi3 = sm2.tile([3, N], f32r, tag="xi3")
                    nc.sync.dma_start(xi3[:], xist_d[:, b * N:(b + 1) * N])
                    x1cm = pcmA.tile([U, N], f32r, tag="x1cm")
                    for np2 in range(KT // 2):
                        pst = ps2.tile([128, 256], f32, tag="tr")
                        for j in range(2):
                            nc.tensor.transpose(
                                pst[:, j * 128:(j + 1) * 128],
                                x1t[:, np2 * 2 + j, :].bitcast(f32), ident[:])
                        nc.scalar.copy(x1cm[:, np2 * 256:(np2 + 1) * 256], pst[:])
                    x2cm = pcmA.tile([U, N], f32r, tag="x2cm")
                    psx = [psAcc.tile([128, 512], f32, tag="acc",
                                      name=f"bx{b}_{ci}") for ci in range(2)]
                    for nt in range(KT):
                        for ci in range(2):
                            nc.tensor.matmul(
                                psx[ci][:], x1t[:, nt, :],
                                s_sb[:, nt, ci * 512:(ci + 1) * 512],
                                start=(nt == 0), stop=(nt == KT - 1))
                    for ci in range(2):
                        c5 = slice(ci * 512, (ci + 1) * 512)
                        nc.vector.scalar_tensor_tensor(
                            x2cm[:, c5], psx[ci][:], 2.0, hcm[:, c5],
                            op0=ALU.mult, op1=ALU.subtract)
                    # gate W-stage: 4 psums (2 ot x 2 ch), k-outer
                    psw = [[psAcc.tile([128, 512], f32, tag="acc",
                                       name=f"bw{b}_{ot}_{ci}")
                            for ci in range(2)] for ot in range(2)]
                    rhs_k = [hcm, x1cm, x2cm]
                    for ot in range(2):
                        o0 = ot * 128
                        for k in range(3):
                            for ci in range(2):
                                c5 = slice(ci * 512, (ci + 1) * 512)
                                nc.tensor.matmul(psw[ot][ci][:], wgh_k(k, o0),
                                                 rhs_k[k][:, c5],
                                                 start=(k == 0), stop=False)
                        for ci in range(2):
                            c5 = slice(ci * 512, (ci + 1) * 512)
                            nc.tensor.matmul(psw[ot][ci][:],
                                             wb[0:3, WGXI + o0:WGXI + o0 + 128],
                                             xi3[:, c5], start=False, stop=True)
                    for ci in range(2):
                        c5 = slice(ci * 512, (ci + 1) * 512)
                        # r
                        sgr = sm2.tile([128, 512], f32, tag="sg")
                        nc.scalar.activation(sgr[:], psw[0][ci][:], AF.Sigmoid,
                                             bias=wb[:, WGBC:WGBC + 1])
                        rhc = sm1.tile([128, 512], f32r, tag="rhc")
                        nc.vector.tensor_mul(rhc[:], sgr[:], hcm[:, c5])
                        nc.gpsimd.dma_start(rhcm_d[:, b, c5], rhc[:])
                        rht = sm2.tile([128, 4, U], f32r, tag="rht")
                        for tp in range(2):
                            pst = ps2.tile([128, 256], f32, tag="tr")
                            for j in range(2):
                                nc.tensor.transpose(
                                    pst[:, j * 128:(j + 1) * 128],
                                    rhc[:, (tp * 2 + j) * 128:
                                        (tp * 2 + j + 1) * 128]
                                    .bitcast(f32), ident[:])
                            nc.vector.tensor_copy(
                                rht[:, tp * 2:tp * 2 + 2, :]
                                .rearrange("p t u -> p (t u)"), pst[:])
                        m0 = ci * 512
                        nc.sync.dma_start(
                            rhnm_d[m0:m0 + 512, b, :]
                            .rearrange("(t n) u -> n t u", t=4), rht[:])
                        # u
                        sgu = sm2.tile([128, 512], f32, tag="sg")
                        nc.scalar.activation(sgu[:], psw[1][ci][:], AF.Sigmoid,
                                             bias=wb[:, WGBC + 1:WGBC + 2])
                        nc.gpsimd.dma_start(ucm_d[:, b, c5], sgu[:])

            # ================= PHASE C: y1 = S @ rh =================
            with nc.named_scope("phaseC"):
                rh0 = st0.tile([128, KT, HW], f32r, tag="big")
                for kt in range(KT):
                    eng = nc.sync if kt % 2 == 0 else nc.scalar
                    eng.dma_start(
                        rh0[:, kt, :],
                        rhnm_d[kt * 128:(kt + 1) * 128, :, :]
                        .rearrange("n b u -> n (b u)"))
                s_apply(rh0, HW, y1_d)

            # ============ PHASE D: cand + new_h + proj ============
            with nc.named_scope("phaseD"):
                for b in range(BC):
                    bc0 = b * 128
                    hcm = pcmB.tile([U, N], f32r, tag="hcm")
                    nc.sync.dma_start(hcm[:], hcm_d[:, b, :])
                    rhcm = pcmC.tile([U, N], f32r, tag="rhcm")
                    nc.sync.dma_start(rhcm[:], rhcm_d[:, b, :])
                    usb = pcmC.tile([U, N], f32, tag="usb")
                    nc.sync.dma_start(usb[:], ucm_d[:, b, :])
                    y1t = pcmA.tile([128, KT, 128], f32r, tag="x1t")
                    nc.sync.dma_start(
                        y1t[:], y1_d[:, bc0:bc0 + 128]
                        .rearrange("(k n) c -> n k c", n=128))
                    xi3 = sm2.tile([3, N], f32r, tag="xi3")
                    nc.sync.dma_start(xi3[:], xist_d[:, b * N:(b + 1) * N])
                    y1cm = pcmA.tile([U, N], f32r, tag="x1cm")
                    for np2 in range(KT // 2):
                        pst = ps2.tile([128, 256], f32, tag="tr")
                        for j in range(2):
                            nc.tensor.transpose(
                                pst[:, j * 128:(j + 1) * 128],
                                y1t[:, np2 * 2 + j, :].bitcast(f32), ident[:])
                        nc.scalar.copy(y1cm[:, np2 * 256:(np2 + 1) * 256], pst[:])
                    y2cm = pcmA.tile([U, N], f32r, tag="x2cm")
                    psx = [psAcc.tile([128, 512], f32, tag="acc",
                                      name=f"dx{b}_{ci}") for ci in range(2)]
                    for nt in range(KT):
                        for ci in range(2):
                            nc.tensor.matmul(
                                psx[ci][:], y1t[:, nt, :],
                                s_sb[:, nt, ci * 512:(ci + 1) * 512],
                                start=(nt == 0), stop=(nt == KT - 1))
                    for ci in range(2):
                        c5 = slice(ci * 512, (ci + 1) * 512)
                        nc.vector.scalar_tensor_tensor(
                            y2cm[:, c5], psx[ci][:], 2.0, rhcm[:, c5],
                            op0=ALU.mult, op1=ALU.subtract)
                    psw = [psAcc.tile([128, 512], f32, tag="acc",
                                      name=f"dw{b}_{ci}") for ci in range(2)]
                    rhs_k = [rhcm, y1cm, y2cm]
                    for k in range(3):
                        for ci in range(2):
                            c5 = slice(ci * 512, (ci + 1) * 512)
                            nc.tensor.matmul(psw[ci][:], wch_k(k),
                                             rhs_k[k][:, c5],
                                             start=(k == 0), stop=False)
                    for ci in range(2):
                        c5 = slice(ci * 512, (ci + 1) * 512)
                        nc.tensor.matmul(psw[ci][:], wb[0:3, WCXI:WCXI + 128],
                                         xi3[:, c5], start=False, stop=True)
                    for ci in range(2):
                        c5 = slice(ci * 512, (ci + 1) * 512)
                        t0 = b * N + ci * 512
                        # z1 = u*h computed before tanh lands
                        z1 = sm1.tile([128, 512], f32, tag="rhc")
                        nc.vector.tensor_mul(z1[:], usb[:, c5], hcm[:, c5])
                        csb = sm2.tile([128, 512], f32, tag="sg")
                        nc.scalar.activation(csb[:], psw[ci][:], AF.Tanh,
                                             bias=wb[:, WCBC:WCBC + 1])
                        # new_h = z1 - (u-1)*c
                        t2 = sm2.tile([128, 512], f32, tag="sg")
                        nc.vector.scalar_tensor_tensor(
                            t2[:], usb[:, c5], 1.0, csb[:],
                            op0=ALU.subtract, op1=ALU.mult)
                        nh = sm1.tile([128, 512], f32r, tag="nh")
                        nc.vector.tensor_sub(nh[:], z1[:], t2[:])
                        nc.gpsimd.dma_start(nh_d[:, b, c5], nh[:])
                        psp = ps2.tile([1, 512], f32, tag="sml")
                        nc.tensor.matmul(psp[:], wb[:, WPW0:WPW0 + 1], nh[:],
                                         start=True, stop=True)
                        po = sm2.tile([1, 512], f32, tag="po")
                        nc.scalar.activation(po[:], psp[:], AF.Identity,
                                             bias=wb[0:1, WPB:WPB + 1])
                        nc.gpsimd.dma_start(proj_d[:, t0:t0 + 512], po[:])

    nc.compile()
    return nc


def _prep(inputs, hidden_state, support, gate_w, gate_b, cand_w, cand_b,
          proj_w, proj_b):
    """Host-side sharding + layout prep. Returns list of per-core in_maps."""
    f = np.float32
    S_T = np.ascontiguousarray(np.asarray(support, f).T)
    h3 = np.asarray(hidden_state, f)[0].reshape(B, N, U)
    xi = np.asarray(inputs, f).reshape(B, N)
    gw = np.asarray(gate_w, f)
    cw = np.asarray(cand_w, f)
    # W rows: flat col index c*3+k ; c=0 is xi, c=1+j is hidden channel j
    hid_rows = 3 * (1 + np.arange(U))
    shared = dict(
        s_t=S_T,
        wgh=np.ascontiguousarray(np.stack([gw[hid_rows + k] for k in range(3)])),
        wgxi=np.ascontiguousarray(gw[0:3]),
        gb=np.asarray(gate_b, f).reshape(1, 2 * U),
        wch=np.ascontiguousarray(np.stack([cw[hid_rows + k] for k in range(3)])),
        wcxi=np.ascontiguousarray(cw[0:3]),
        cb=np.asarray(cand_b, f).reshape(1, U),
        pw=np.asarray(proj_w, f).reshape(U, 1),
        pb=np.asarray(proj_b, f).reshape(1, 1),
    )
    maps = []
    for c in range(NCORES):
        b0 = c * BC
        hs = h3[b0:b0 + BC]                                       # [BC, N, U]
        h_nm = np.ascontiguousarray(hs.transpose(1, 0, 2)).reshape(N, BC * U)
        h_cm = np.ascontiguousarray(hs.transpose(2, 0, 1))        # [U, BC, N]
        xib = np.ascontiguousarray(xi[b0:b0 + BC])                # [BC, N]
        maps.append(dict(
            shared,
            h_nm=h_nm, h_cm=h_cm,
            xi_nm=np.ascontiguousarray(xib.T),
            xi_bm=xib,
        ))
    return maps


def _gather(results):
    out = np.empty((B, N * OD), np.float32)
    new_h = np.empty((B, N * U), np.float32)
    for c in range(NCORES):
        b0 = c * BC
        r = results[c]
        out[b0:b0 + BC] = r["out_proj"].reshape(BC, N)
        new_h[b0:b0 + BC] = r["nh_cm"].transpose(1, 2, 0).reshape(BC, N * U)
    return out, new_h[None]


def kernel(**inputs):
    from concourse.bass_utils import run_bass_kernel_spmd

    if "nc" not in _cache:
        _cache["nc"] = _build()
    nc = _cache["nc"]
    maps = _prep(**inputs)
    res = run_bass_kernel_spmd(nc, maps, core_ids=list(range(NCORES)),
                               trace=False)
    _cache["last_res"] = res
    return _gather(res.results)


# revision 12
# speedup vs baseline: 1.4046x; 1.0695x over previous
"""DCRNN decoder cell (gnn_message_passing) on 8 Trainium2 cores.

Data-parallel over batch B=128 -> 16 batches/core; support matrix and
weights replicated; no collectives.

Per core (Bc=16):
  x0 = concat([xi, h]); x1 = S x0; x2 = 2 S x1 - x0     (Chebyshev)
  gate = sigmoid(sum_k Xk @ Wg_k + bg) -> r, u
  rh = r*h; y1 = S rh; y2 = 2 S y1 - rh
  c = tanh(sum_k Yk @ Wc_k + bc)
  new_h = u h + (1-u) c ; out = new_h @ pw + pb

Layouts: node-major [n, (b,u)] for Chebyshev; channel-major [u, (b,n)]
for the W-stages (out_T[o,tok] = sum_c Wk[c,o] Xk_cm[c,tok], bias and
the xi channel folded in as K=1 / K=3 matmuls).  x2_cm is computed
directly as (2 S x1 - x0)_cm via lhsT=x1_nm strips, rhs=S^T: the matmul
performs the S-apply and the cm-transpose at once.  x1_cm comes from PE
transposes of x1_nm strips.  All matmuls in float32r (full PE rate).
"""
import numpy as np

import concourse.bacc as bacc
import concourse.tile as tile
from concourse import mybir
from concourse.masks import make_identity

f32 = mybir.dt.float32
f32r = mybir.dt.float32r
AF = mybir.ActivationFunctionType
ALU = mybir.AluOpType

B, N, U, OD = 128, 1024, 128, 1
NCORES = 8
BC = B // NCORES            # 16 batches per core
KT = N // 128               # 8 node tiles
TOK = BC * N                # 16384 tokens per core
HW = BC * U                 # 2048 hidden cols
XW = HW + BC                # 2064 = hidden + xi cols

# packed small-weights tile column offsets (f32r [128, 1792])
WGH0 = 0            # [128, 3*256] gate hidden weights, c-major
WCH0 = 768          # [128, 3*128] cand hidden weights
WPW0 = WCH0 + 384   # [128, 1] proj weight
WROW = WPW0 + 1     # row-tensors region (partitions 0..2):
#   wgxi [3, 256] @ WROW, wcxi [3, 128] @ +256, gb [1, 256] @ +384 (p0),
#   cb [1,128] @ +640 (p0), pb [1,1] @ +768 (p0), ones [1,512] @ +769 (p0)
WGXI = WROW
WCXI = WROW + 256
WGBC = WCXI + 128   # [128, 2] gate bias as columns (r, u)
WCBC = WGBC + 2     # [128, 1] cand bias column
WPB = WCBC + 1      # [1, 1] proj bias
WBIGW = WPB + 1

_cache = {}


def _build():
    nc = bacc.Bacc("TRN2", target_bir_lowering=False, debug=False)

    s_d = nc.dram_tensor("s_t", [N, N], f32r, kind="ExternalInput").ap()
    hnm_d = nc.dram_tensor("h_nm", [N, BC * U], f32r, kind="ExternalInput").ap()
    hcm_d = nc.dram_tensor("h_cm", [U, BC, N], f32r, kind="ExternalInput").ap()
    xinm_d = nc.dram_tensor("xi_nm", [N, BC], f32r, kind="ExternalInput").ap()
    xibm_d = nc.dram_tensor("xi_bm", [BC, N], f32r, kind="ExternalInput").ap()
    wgh_d = nc.dram_tensor("wgh", [3, U, 2 * U], f32r, kind="ExternalInput").ap()
    wgxi_d = nc.dram_tensor("wgxi", [3, 2 * U], f32r, kind="ExternalInput").ap()
    gb_d = nc.dram_tensor("gb", [1, 2 * U], f32r, kind="ExternalInput").ap()
    wch_d = nc.dram_tensor("wch", [3, U, U], f32r, kind="ExternalInput").ap()
    wcxi_d = nc.dram_tensor("wcxi", [3, U], f32r, kind="ExternalInput").ap()
    cb_d = nc.dram_tensor("cb", [1, U], f32r, kind="ExternalInput").ap()
    pw_d = nc.dram_tensor("pw", [U, 1], f32r, kind="ExternalInput").ap()
    pb_d = nc.dram_tensor("pb", [1, 1], f32r, kind="ExternalInput").ap()

    proj_d = nc.dram_tensor("out_proj", [1, TOK], f32, kind="ExternalOutput").ap()
    nh_d = nc.dram_tensor("nh_cm", [U, BC, N], f32r, kind="ExternalOutput").ap()

    x1_d = nc.dram_tensor("x1_nm", [N, XW], f32r, kind="Internal").ap()
    y1_d = nc.dram_tensor("y1_nm", [N, HW], f32r, kind="Internal").ap()
    rhnm_d = nc.dram_tensor("rh_nm", [N, BC, U], f32r, kind="Internal").ap()
    rhcm_d = nc.dram_tensor("rh_cm", [U, BC, N], f32r, kind="Internal").ap()
    ucm_d = nc.dram_tensor("u_cm", [U, BC, N], f32, kind="Internal").ap()
    xist_d = nc.dram_tensor("xi_stack", [3, TOK], f32r, kind="Internal").ap()

    with tile.TileContext(nc) as tc:
        with (
            tc.tile_pool(name="consts", bufs=1) as consts,
            tc.tile_pool(name="stage1", bufs=1) as st1,
            tc.tile_pool(name="stage0", bufs=1) as st0,
            tc.tile_pool(name="pcmA", bufs=2) as pcmA,
            tc.tile_pool(name="pcmB", bufs=2) as pcmB,
            tc.tile_pool(name="pcmC", bufs=2) as pcmC,
            tc.tile_pool(name="sm2", bufs=2) as sm2,
            tc.tile_pool(name="sm1", bufs=2) as sm1,
            tc.tile_pool(name="ps", bufs=4, space="PSUM") as psAcc,
            tc.tile_pool(name="ps2", bufs=2, space="PSUM") as ps2,
        ):
            # ---------------- constants ----------------
            ident = consts.tile([128, 128], f32)
            make_identity(nc, ident)
            s_sb = consts.tile([128, KT, N], f32r)
            for kt in range(KT):
                nc.sync.dma_start(s_sb[:, kt, :], s_d[kt * 128:(kt + 1) * 128, :])
            wb = consts.tile([128, WBIGW], f32r)
            nc.sync.dma_start(
                wb[:, WGH0:WGH0 + 768].rearrange("c (k o) -> c k o", k=3),
                wgh_d[:, :, :].rearrange("k c o -> c k o"))
            nc.sync.dma_start(
                wb[:, WCH0:WCH0 + 384].rearrange("c (k o) -> c k o", k=3),
                wch_d[:, :, :].rearrange("k c o -> c k o"))
            nc.sync.dma_start(wb[:, WPW0:WPW0 + 1], pw_d[:, :])
            nc.sync.dma_start(wb[0:3, WGXI:WGXI + 256], wgxi_d[:, :])
            nc.sync.dma_start(wb[0:3, WCXI:WCXI + 128], wcxi_d[:, :])
            nc.sync.dma_start(wb[:, WGBC:WGBC + 2],
                              gb_d[:, :].rearrange("one (t c) -> c (one t)", t=2))
            nc.sync.dma_start(wb[:, WCBC:WCBC + 1],
                              cb_d[:, :].rearrange("one c -> c one"))
            nc.sync.dma_start(wb[0:1, WPB:WPB + 1], pb_d[:, :])

            def wgh_k(k, o0):
                return wb[:, WGH0 + k * 256 + o0:WGH0 + k * 256 + o0 + 128]

            def wch_k(k):
                return wb[:, WCH0 + k * 128:WCH0 + (k + 1) * 128]

            # -------- helper: node-major S-apply, resident source --------
            def s_apply(src_res, width, dst_dram, dst_c0=0, pfx=""):
                """dst_dram[mt-block, :width] = S @ src_res ([128, KT, width]).
                mt/kt-outer so each stationary S-tile serves all chunks."""
                nch = width // 512
                tail = width - nch * 512
                for mt in range(KT):
                    pss = [psAcc.tile([128, 512], f32, tag="acc",
                                      name=f"apl{mt}_{ci}")
                           for ci in range(nch)]
                    pxi = (ps2.tile([128, 128], f32, tag="tr",
                                    name=f"apx{pfx}{mt}")
                           if tail else None)
                    for kt in range(KT):
                        for ci in range(nch):
                            nc.tensor.matmul(
                                pss[ci][:], s_sb[:, kt, mt * 128:(mt + 1) * 128],
                                src_res[:, kt, ci * 512:(ci + 1) * 512],
                                start=(kt == 0), stop=(kt == KT - 1))
                        if tail:
                            nc.tensor.matmul(
                                pxi[:, 0:tail],
                                s_sb[:, kt, mt * 128:(mt + 1) * 128],
                                src_res[:, kt, nch * 512:width],
                                start=(kt == 0), stop=(kt == KT - 1))
                    for ci in range(nch):
                        so = sm2.tile([128, 512], f32r, tag="sg",
                                      name=f"so{pfx}{mt}_{ci}")
                        nc.vector.tensor_copy(so[:], pss[ci][:])
                        nc.sync.dma_start(
                            dst_dram[mt * 128:(mt + 1) * 128,
                                     dst_c0 + ci * 512:dst_c0 + (ci + 1) * 512],
                            so[:])
                    if tail:
                        sot = sm2.tile([128, tail], f32r, tag="po",
                                       name=f"sot{pfx}{mt}")
                        nc.vector.tensor_copy(sot[:], pxi[:, 0:tail])
                        nc.sync.dma_start(
                            dst_dram[mt * 128:(mt + 1) * 128,
                                     dst_c0 + nch * 512:dst_c0 + width],
                            sot[:])

            # ================= PHASE A: x1 = S @ x0 =================
            with nc.named_scope("phaseA"):
                x0 = st0.tile([128, KT, XW], f32r, tag="big")
                for kt in range(KT):
                    eng = nc.sync if kt % 2 == 0 else nc.scalar
                    eng.dma_start(x0[:, kt, 0:HW],
                                  hnm_d[kt * 128:(kt + 1) * 128, :])
                    eng.dma_start(x0[:, kt, HW:XW],
                                  xinm_d[kt * 128:(kt + 1) * 128, :])
                s_apply(x0, XW, x1_d)

                # xi chebyshev rows -> DRAM xi_stack
                with tc.tile_pool(name="xip", bufs=1) as xip:
                    xibm = xip.tile([BC, N], f32r)
                    nc.sync.dma_start(xibm[:], xibm_d[:, :])
                    nc.sync.dma_start(xist_d[0:1, :], xibm[:, :])
                    x1xit = xip.tile([128, KT, BC], f32r)
                    nc.sync.dma_start(
                        x1xit[:], x1_d[:, HW:XW].rearrange("(k n) c -> n k c",
                                                           n=128))
                    x1xi = xip.tile([BC, N], f32r)
                    x2xi = xip.tile([BC, N], f32r)
                    for ch in range(2):
                        c5 = slice(ch * 512, (ch + 1) * 512)
                        ps = ps2.tile([BC, 512], f32, tag="sml")
                        for kt in range(KT):
                            nc.tensor.matmul(ps[:], x0[:, kt, HW:XW],
                                             s_sb[:, kt, c5],
                                             start=(kt == 0), stop=(kt == KT - 1))
                        nc.vector.tensor_copy(x1xi[:, c5], ps[:])
                        psb = ps2.tile([BC, 512], f32, tag="sml")
                        for nt in range(KT):
                            nc.tensor.matmul(psb[:], x1xit[:, nt, :],
                                             s_sb[:, nt, c5],
                                             start=(nt == 0), stop=(nt == KT - 1))
                        nc.vector.scalar_tensor_tensor(
                            x2xi[:, c5], psb[:], 2.0, xibm[:, c5],
                            op0=ALU.mult, op1=ALU.subtract)
                    nc.sync.dma_start(xist_d[1:2, :], x1xi[:, :])
                    nc.sync.dma_start(xist_d[2:3, :], x2xi[:, :])

            # ============ PHASE B: per-batch gate pipeline ============
            def phase_b(b_range):
                for b in b_range:
                    bc0 = b * 128
                    hcm = pcmB.tile([U, N], f32r, tag="hcm")
                    nc.sync.dma_start(hcm[:], hcm_d[:, b, :])
                    x1t = pcmA.tile([128, KT, 128], f32r, tag="x1t")
                    nc.sync.dma_start(
                        x1t[:], x1_d[:, bc0:bc0 + 128]
                        .rearrange("(k n) c -> n k c", n=128))
                    xi3 = sm2.tile([3, N], f32r, tag="xi3")
                    nc.sync.dma_start(xi3[:], xist_d[:, b * N:(b + 1) * N])
                    x1cm = pcmA.tile([U, N], f32r, tag="x1cm")
                    for np2 in range(KT // 2):
                        pst = ps2.tile([128, 256], f32, tag="tr")
                        for j in range(2):
                            nc.tensor.transpose(
                                pst[:, j * 128:(j + 1) * 128],
                                x1t[:, np2 * 2 + j, :].bitcast(f32), ident[:])
                        nc.scalar.copy(x1cm[:, np2 * 256:(np2 + 1) * 256], pst[:])
                    x2cm = pcmA.tile([U, N], f32r, tag="x2cm")
                    psx = [psAcc.tile([128, 512], f32, tag="acc",
                                      name=f"bx{b}_{ci}") for ci in range(2)]
                    for nt in range(KT):
                        for ci in range(2):
                            nc.tensor.matmul(
                                psx[ci][:], x1t[:, nt, :],
                                s_sb[:, nt, ci * 512:(ci + 1) * 512],
                                start=(nt == 0), stop=(nt == KT - 1))
                    for ci in range(2):
                        c5 = slice(ci * 512, (ci + 1) * 512)
                        nc.vector.scalar_tensor_tensor(
                            x2cm[:, c5], psx[ci][:], 2.0, hcm[:, c5],
                            op0=ALU.mult, op1=ALU.subtract)
                    # gate W-stage: 4 psums (2 ot x 2 ch), k-outer
                    psw = [[psAcc.tile([128, 512], f32, tag="acc",
                                       name=f"bw{b}_{ot}_{ci}")
                            for ci in range(2)] for ot in range(2)]
                    rhs_k = [hcm, x1cm, x2cm]
                    for ot in range(2):
                        o0 = ot * 128
                        for k in range(3):
                            for ci in range(2):
                                c5 = slice(ci * 512, (ci + 1) * 512)
                                nc.tensor.matmul(psw[ot][ci][:], wgh_k(k, o0),
                                                 rhs_k[k][:, c5],
                                                 start=(k == 0), stop=False)
                        for ci in range(2):
                            c5 = slice(ci * 512, (ci + 1) * 512)
                            nc.tensor.matmul(psw[ot][ci][:],
                                             wb[0:3, WGXI + o0:WGXI + o0 + 128],
                                             xi3[:, c5], start=False, stop=True)
                    for ci in range(2):
                        c5 = slice(ci * 512, (ci + 1) * 512)
                        # r
                        sgr = sm2.tile([128, 512], f32, tag="sg")
                        nc.scalar.activation(sgr[:], psw[0][ci][:], AF.Sigmoid,
                                             bias=wb[:, WGBC:WGBC + 1])
                        rhc = sm1.tile([128, 512], f32r, tag="rhc")
                        nc.vector.tensor_mul(rhc[:], sgr[:], hcm[:, c5])
                        nc.gpsimd.dma_start(rhcm_d[:, b, c5], rhc[:])
                        rht = sm2.tile([128, 4, U], f32r, tag="rht")
                        for tp in range(2):
                            pst = ps2.tile([128, 256], f32, tag="tr")
                            for j in range(2):
                                nc.tensor.transpose(
                                    pst[:, j * 128:(j + 1) * 128],
                                    rhc[:, (tp * 2 + j) * 128:
                                        (tp * 2 + j + 1) * 128]
                                    .bitcast(f32), ident[:])
                            nc.vector.tensor_copy(
                                rht[:, tp * 2:tp * 2 + 2, :]
                                .rearrange("p t u -> p (t u)"), pst[:])
                        m0 = ci * 512
                        nc.sync.dma_start(
                            rhnm_d[m0:m0 + 512, b, :]
                            .rearrange("(t n) u -> n t u", t=4), rht[:])
                        # u
                        sgu = sm2.tile([128, 512], f32, tag="sg")
                        nc.scalar.activation(sgu[:], psw[1][ci][:], AF.Sigmoid,
                                             bias=wb[:, WGBC + 1:WGBC + 2])
                        nc.gpsimd.dma_start(ucm_d[:, b, c5], sgu[:])

            # ================= PHASE C: y1 = S @ rh =================
            def phase_c(g):
                gw = HW // 2
                rh0 = st0.tile([128, KT, gw], f32r, tag="big",
                               name=f"rh0_{g}")
                for kt in range(KT):
                    eng = nc.sync if kt % 2 == 0 else nc.scalar
                    eng.dma_start(
                        rh0[:, kt, :],
                        rhnm_d[kt * 128:(kt + 1) * 128,
                               g * (BC // 2):(g + 1) * (BC // 2), :]
                        .rearrange("n b u -> n (b u)"))
                s_apply(rh0, gw, y1_d, dst_c0=g * gw, pfx=f"c{g}")

            # ============ PHASE D: cand + new_h + proj ============
            def phase_d(b_range):
                for b in b_range:
                    bc0 = b * 128
                    hcm = pcmB.tile([U, N], f32r, tag="hcm")
                    nc.sync.dma_start(hcm[:], hcm_d[:, b, :])
                    rhcm = pcmC.tile([U, N], f32r, tag="rhcm")
                    nc.sync.dma_start(rhcm[:], rhcm_d[:, b, :])
                    usb = pcmC.tile([U, N], f32, tag="usb")
                    nc.sync.dma_start(usb[:], ucm_d[:, b, :])
                    y1t = pcmA.tile([128, KT, 128], f32r, tag="x1t")
                    nc.sync.dma_start(
                        y1t[:], y1_d[:, bc0:bc0 + 128]
                        .rearrange("(k n) c -> n k c", n=128))
                    xi3 = sm2.tile([3, N], f32r, tag="xi3")
                    nc.sync.dma_start(xi3[:], xist_d[:, b * N:(b + 1) * N])
                    y1cm = pcmA.tile([U, N], f32r, tag="x1cm")
                    for np2 in range(KT // 2):
                        pst = ps2.tile([128, 256], f32, tag="tr")
                        for j in range(2):
                            nc.tensor.transpose(
                                pst[:, j * 128:(j + 1) * 128],
                                y1t[:, np2 * 2 + j, :].bitcast(f32), ident[:])
                        nc.scalar.copy(y1cm[:, np2 * 256:(np2 + 1) * 256], pst[:])
                    y2cm = pcmA.tile([U, N], f32r, tag="x2cm")
                    psx = [psAcc.tile([128, 512], f32, tag="acc",
                                      name=f"dx{b}_{ci}") for ci in range(2)]
                    for nt in range(KT):
                        for ci in range(2):
                            nc.tensor.matmul(
                                psx[ci][:], y1t[:, nt, :],
                                s_sb[:, nt, ci * 512:(ci + 1) * 512],
                                start=(nt == 0), stop=(nt == KT - 1))
                    for ci in range(2):
                        c5 = slice(ci * 512, (ci + 1) * 512)
                        nc.vector.scalar_tensor_tensor(
                            y2cm[:, c5], psx[ci][:], 2.0, rhcm[:, c5],
                            op0=ALU.mult, op1=ALU.subtract)
                    psw = [psAcc.tile([128, 512], f32, tag="acc",
                                      name=f"dw{b}_{ci}") for ci in range(2)]
                    rhs_k = [rhcm, y1cm, y2cm]
                    for k in range(3):
                        for ci in range(2):
                            c5 = slice(ci * 512, (ci + 1) * 512)
                            nc.tensor.matmul(psw[ci][:], wch_k(k),
                                             rhs_k[k][:, c5],
                                             start=(k == 0), stop=False)
                    for ci in range(2):
                        c5 = slice(ci * 512, (ci + 1) * 512)
                        nc.tensor.matmul(psw[ci][:], wb[0:3, WCXI:WCXI + 128],
                                         xi3[:, c5], start=False, stop=True)
                    for ci in range(2):
                        c5 = slice(ci * 512, (ci + 1) * 512)
                        t0 = b * N + ci * 512
                        # z1 = u*h computed before tanh lands
                        z1 = sm1.tile([128, 512], f32, tag="rhc")
                        nc.vector.tensor_mul(z1[:], usb[:, c5], hcm[:, c5])
                        csb = sm2.tile([128, 512], f32, tag="sg")
                        nc.scalar.activation(csb[:], psw[ci][:], AF.Tanh,
                                             bias=wb[:, WCBC:WCBC + 1])
                        # new_h = z1 - (u-1)*c
                        t2 = sm2.tile([128, 512], f32, tag="sg")
                        nc.vector.scalar_tensor_tensor(
                            t2[:], usb[:, c5], 1.0, csb[:],
                            op0=ALU.subtract, op1=ALU.mult)
                        nh = sm1.tile([128, 512], f32r, tag="nh")
                        nc.vector.tensor_sub(nh[:], z1[:], t2[:])
                        nc.gpsimd.dma_start(nh_d[:, b, c5], nh[:])
                        psp = ps2.tile([1, 512], f32, tag="sml")
                        nc.tensor.matmul(psp[:], wb[:, WPW0:WPW0 + 1], nh[:],
                                         start=True, stop=True)
                        po = sm2.tile([1, 512], f32, tag="po")
                        nc.scalar.activation(po[:], psp[:], AF.Identity,
                                             bias=wb[0:1, WPB:WPB + 1])
                        nc.gpsimd.dma_start(proj_d[:, t0:t0 + 512], po[:])

            with nc.named_scope("phaseBCD"):
                phase_b(range(0, BC // 2))
                phase_b(range(BC // 2, BC))
                phase_c(0)
                phase_c(1)
                phase_d(range(0, BC // 2))
                phase_d(range(BC // 2, BC))

    nc.compile()
    return nc


def _prep(inputs, hidden_state, support, gate_w, gate_b, cand_w, cand_b,
          proj_w, proj_b):
    """Host-side sharding + layout prep. Returns list of per-core in_maps."""
    f = np.float32
    S_T = np.ascontiguousarray(np.asarray(support, f).T)
    h3 = np.asarray(hidden_state, f)[0].reshape(B, N, U)
    xi = np.asarray(inputs, f).reshape(B, N)
    gw = np.asarray(gate_w, f)
    cw = np.asarray(cand_w, f)
    # W rows: flat col index c*3+k ; c=0 is xi, c=1+j is hidden channel j
    hid_rows = 3 * (1 + np.arange(U))
    shared = dict(
        s_t=S_T,
        wgh=np.ascontiguousarray(np.stack([gw[hid_rows + k] for k in range(3)])),
        wgxi=np.ascontiguousarray(gw[0:3]),
        gb=np.asarray(gate_b, f).reshape(1, 2 * U),
        wch=np.ascontiguousarray(np.stack([cw[hid_rows + k] for k in range(3)])),
        wcxi=np.ascontiguousarray(cw[0:3]),
        cb=np.asarray(cand_b, f).reshape(1, U),
        pw=np.asarray(proj_w, f).reshape(U, 1),
        pb=np.asarray(proj_b, f).reshape(1, 1),
    )
    maps = []
    for c in range(NCORES):
        b0 = c * BC
        hs = h3[b0:b0 + BC]                                       # [BC, N, U]
        h_nm = np.ascontiguousarray(hs.transpose(1, 0, 2)).reshape(N, BC * U)
        h_cm = np.ascontiguousarray(hs.transpose(2, 0, 1))        # [U, BC, N]
        xib = np.ascontiguousarray(xi[b0:b0 + BC])                # [BC, N]
        maps.append(dict(
            shared,
            h_nm=h_nm, h_cm=h_cm,
            xi_nm=np.ascontiguousarray(xib.T),
            xi_bm=xib,
        ))
    return maps


def _gather(results):
    out = np.empty((B, N * OD), np.float32)
    new_h = np.empty((B, N * U), np.float32)
    for c in range(NCORES):
        b0 = c * BC
        r = results[c]
        out[b0:b0 + BC] = r["out_proj"].reshape(BC, N)
        new_h[b0:b0 + BC] = r["nh_cm"].transpose(1, 2, 0).reshape(BC, N * U)
    return out, new_h[None]


def kernel(**inputs):
    from concourse.bass_utils import run_bass_kernel_spmd

    if "nc" not in _cache:
        _cache["nc"] = _build()
    nc = _cache["nc"]
    maps = _prep(**inputs)
    res = run_bass_kernel_spmd(nc, maps, core_ids=list(range(NCORES)),
                               trace=False)
    _cache["last_res"] = res
    return _gather(res.results)


# revision 17
# speedup vs baseline: 1.4074x; 1.0020x over previous
"""DCRNN decoder cell (gnn_message_passing) on 8 Trainium2 cores.

Data-parallel over batch B=128 -> 16 batches/core; support matrix and
weights replicated; no collectives.

Per core (Bc=16):
  x0 = concat([xi, h]); x1 = S x0; x2 = 2 S x1 - x0     (Chebyshev)
  gate = sigmoid(sum_k Xk @ Wg_k + bg) -> r, u
  rh = r*h; y1 = S rh; y2 = 2 S y1 - rh
  c = tanh(sum_k Yk @ Wc_k + bc)
  new_h = u h + (1-u) c ; out = new_h @ pw + pb

Layouts: node-major [n, (b,u)] for Chebyshev; channel-major [u, (b,n)]
for the W-stages (out_T[o,tok] = sum_c Wk[c,o] Xk_cm[c,tok], bias and
the xi channel folded in as K=1 / K=3 matmuls).  x2_cm is computed
directly as (2 S x1 - x0)_cm via lhsT=x1_nm strips, rhs=S^T: the matmul
performs the S-apply and the cm-transpose at once.  x1_cm comes from PE
transposes of x1_nm strips.  All matmuls in float32r (full PE rate).
"""
import numpy as np

import concourse.bacc as bacc
import concourse.tile as tile
from concourse import mybir
from concourse.masks import make_identity

f32 = mybir.dt.float32
f32r = mybir.dt.float32r
AF = mybir.ActivationFunctionType
ALU = mybir.AluOpType

B, N, U, OD = 128, 1024, 128, 1
NCORES = 8
BC = B // NCORES            # 16 batches per core
KT = N // 128               # 8 node tiles
TOK = BC * N                # 16384 tokens per core
HW = BC * U                 # 2048 hidden cols
XW = HW + BC                # 2064 = hidden + xi cols

# packed small-weights tile column offsets (f32r [128, 1792])
WGH0 = 0            # [128, 3*256] gate hidden weights, c-major
WCH0 = 768          # [128, 3*128] cand hidden weights
WPW0 = WCH0 + 384   # [128, 1] proj weight
WROW = WPW0 + 1     # row-tensors region (partitions 0..2):
#   wgxi [3, 256] @ WROW, wcxi [3, 128] @ +256, gb [1, 256] @ +384 (p0),
#   cb [1,128] @ +640 (p0), pb [1,1] @ +768 (p0), ones [1,512] @ +769 (p0)
WGXI = WROW
WCXI = WROW + 256
WGBC = WCXI + 128   # [128, 2] gate bias as columns (r, u)
WCBC = WGBC + 2     # [128, 1] cand bias column
WPB = WCBC + 1      # [1, 1] proj bias
WBIGW = WPB + 1

_cache = {}


def _build():
    nc = bacc.Bacc("TRN2", target_bir_lowering=False, debug=False)

    s_d = nc.dram_tensor("s_t", [N, N], f32r, kind="ExternalInput").ap()
    hnm_d = nc.dram_tensor("h_nm", [N, BC * U], f32r, kind="ExternalInput").ap()
    hcm_d = nc.dram_tensor("h_cm", [U, BC, N], f32r, kind="ExternalInput").ap()
    xinm_d = nc.dram_tensor("xi_nm", [N, BC], f32r, kind="ExternalInput").ap()
    xibm_d = nc.dram_tensor("xi_bm", [BC, N], f32r, kind="ExternalInput").ap()
    wgh_d = nc.dram_tensor("wgh", [3, U, 2 * U], f32r, kind="ExternalInput").ap()
    wgxi_d = nc.dram_tensor("wgxi", [3, 2 * U], f32r, kind="ExternalInput").ap()
    gb_d = nc.dram_tensor("gb", [1, 2 * U], f32r, kind="ExternalInput").ap()
    wch_d = nc.dram_tensor("wch", [3, U, U], f32r, kind="ExternalInput").ap()
    wcxi_d = nc.dram_tensor("wcxi", [3, U], f32r, kind="ExternalInput").ap()
    cb_d = nc.dram_tensor("cb", [1, U], f32r, kind="ExternalInput").ap()
    pw_d = nc.dram_tensor("pw", [U, 1], f32r, kind="ExternalInput").ap()
    pb_d = nc.dram_tensor("pb", [1, 1], f32r, kind="ExternalInput").ap()

    proj_d = nc.dram_tensor("out_proj", [1, TOK], f32, kind="ExternalOutput").ap()
    nh_d = nc.dram_tensor("nh_cm", [U, BC, N], f32r, kind="ExternalOutput").ap()

    x1_d = nc.dram_tensor("x1_nm", [N, XW], f32r, kind="Internal").ap()
    y1_d = nc.dram_tensor("y1_nm", [N, HW], f32r, kind="Internal").ap()
    rhnm_d = nc.dram_tensor("rh_nm", [N, BC, U], f32r, kind="Internal").ap()
    rhcm_d = nc.dram_tensor("rh_cm", [U, BC, N], f32r, kind="Internal").ap()
    ucm_d = nc.dram_tensor("u_cm", [U, BC, N], f32, kind="Internal").ap()
    xist_d = nc.dram_tensor("xi_stack", [3, TOK], f32r, kind="Internal").ap()

    with tile.TileContext(nc) as tc:
        with (
            tc.tile_pool(name="consts", bufs=1) as consts,
            tc.tile_pool(name="stage1", bufs=1) as st1,
            tc.tile_pool(name="stage0", bufs=1) as st0,
            tc.tile_pool(name="pcmA", bufs=2) as pcmA,
            tc.tile_pool(name="pcmB", bufs=2) as pcmB,
            tc.tile_pool(name="pcmC", bufs=2) as pcmC,
            tc.tile_pool(name="sm2", bufs=2) as sm2,
            tc.tile_pool(name="sm1", bufs=2) as sm1,
            tc.tile_pool(name="ps", bufs=4, space="PSUM") as psAcc,
            tc.tile_pool(name="ps2", bufs=2, space="PSUM") as ps2,
        ):
            # ---------------- constants ----------------
            ident = consts.tile([128, 128], f32)
            make_identity(nc, ident)
            s_sb = consts.tile([128, KT, N], f32r)
            for kt in range(KT):
                nc.sync.dma_start(s_sb[:, kt, :], s_d[kt * 128:(kt + 1) * 128, :])
            wb = consts.tile([128, WBIGW], f32r)
            nc.sync.dma_start(
                wb[:, WGH0:WGH0 + 768].rearrange("c (k o) -> c k o", k=3),
                wgh_d[:, :, :].rearrange("k c o -> c k o"))
            nc.sync.dma_start(
                wb[:, WCH0:WCH0 + 384].rearrange("c (k o) -> c k o", k=3),
                wch_d[:, :, :].rearrange("k c o -> c k o"))
            nc.sync.dma_start(wb[:, WPW0:WPW0 + 1], pw_d[:, :])
            nc.sync.dma_start(wb[0:3, WGXI:WGXI + 256], wgxi_d[:, :])
            nc.sync.dma_start(wb[0:3, WCXI:WCXI + 128], wcxi_d[:, :])
            nc.sync.dma_start(wb[:, WGBC:WGBC + 2],
                              gb_d[:, :].rearrange("one (t c) -> c (one t)", t=2))
            nc.sync.dma_start(wb[:, WCBC:WCBC + 1],
                              cb_d[:, :].rearrange("one c -> c one"))
            nc.sync.dma_start(wb[0:1, WPB:WPB + 1], pb_d[:, :])

            def wgh_k(k, o0):
                return wb[:, WGH0 + k * 256 + o0:WGH0 + k * 256 + o0 + 128]

            def wch_k(k):
                return wb[:, WCH0 + k * 128:WCH0 + (k + 1) * 128]

            # -------- helper: node-major S-apply, resident source --------
            def s_apply(src_res, width, dst_dram, dst_c0=0, pfx=""):
                """dst_dram[mt-block, :width] = S @ src_res ([128, KT, width]).
                mt/kt-outer so each stationary S-tile serves all chunks."""
                nch = width // 512
                tail = width - nch * 512
                for mt in range(KT):
                    pss = [psAcc.tile([128, 512], f32, tag="acc",
                                      name=f"apl{mt}_{ci}")
                           for ci in range(nch)]
                    pxi = (ps2.tile([128, 128], f32, tag="tr",
                                    name=f"apx{pfx}{mt}")
                           if tail else None)
                    for kt in range(KT):
                        for ci in range(nch):
                            nc.tensor.matmul(
                                pss[ci][:], s_sb[:, kt, mt * 128:(mt + 1) * 128],
                                src_res[:, kt, ci * 512:(ci + 1) * 512],
                                start=(kt == 0), stop=(kt == KT - 1))
                        if tail:
                            nc.tensor.matmul(
                                pxi[:, 0:tail],
                                s_sb[:, kt, mt * 128:(mt + 1) * 128],
                                src_res[:, kt, nch * 512:width],
                                start=(kt == 0), stop=(kt == KT - 1))
                    for ci in range(nch):
                        so = sm2.tile([128, 512], f32r, tag="sg",
                                      name=f"so{pfx}{mt}_{ci}")
                        nc.vector.tensor_copy(so[:], pss[ci][:])
                        nc.sync.dma_start(
                            dst_dram[mt * 128:(mt + 1) * 128,
                                     dst_c0 + ci * 512:dst_c0 + (ci + 1) * 512],
                            so[:])
                    if tail:
                        sot = sm2.tile([128, tail], f32r, tag="po",
                                       name=f"sot{pfx}{mt}")
                        nc.vector.tensor_copy(sot[:], pxi[:, 0:tail])
                        nc.sync.dma_start(
                            dst_dram[mt * 128:(mt + 1) * 128,
                                     dst_c0 + nch * 512:dst_c0 + width],
                            sot[:])

            # ================= PHASE A: x1 = S @ x0 =================
            with nc.named_scope("phaseA"):
                x0 = st0.tile([128, KT, XW], f32r, tag="big")
                for kt in range(KT):
                    eng = nc.sync if kt % 2 == 0 else nc.scalar
                    eng.dma_start(x0[:, kt, 0:HW],
                                  hnm_d[kt * 128:(kt + 1) * 128, :])
                    eng.dma_start(x0[:, kt, HW:XW],
                                  xinm_d[kt * 128:(kt + 1) * 128, :])
                s_apply(x0, XW, x1_d)

                # xi chebyshev rows -> DRAM xi_stack
                with tc.tile_pool(name="xip", bufs=1) as xip:
                    xibm = xip.tile([BC, N], f32r)
                    nc.sync.dma_start(xibm[:], xibm_d[:, :])
                    nc.sync.dma_start(xist_d[0:1, :], xibm[:, :])
                    x1xit = xip.tile([128, KT, BC], f32r)
                    nc.sync.dma_start(
                        x1xit[:], x1_d[:, HW:XW].rearrange("(k n) c -> n k c",
                                                           n=128))
                    x1xi = xip.tile([BC, N], f32r)
                    x2xi = xip.tile([BC, N], f32r)
                    for ch in range(2):
                        c5 = slice(ch * 512, (ch + 1) * 512)
                        ps = ps2.tile([BC, 512], f32, tag="sml")
                        for kt in range(KT):
                            nc.tensor.matmul(ps[:], x0[:, kt, HW:XW],
                                             s_sb[:, kt, c5],
                                             start=(kt == 0), stop=(kt == KT - 1))
                        nc.vector.tensor_copy(x1xi[:, c5], ps[:])
                        psb = ps2.tile([BC, 512], f32, tag="sml")
                        for nt in range(KT):
                            nc.tensor.matmul(psb[:], x1xit[:, nt, :],
                                             s_sb[:, nt, c5],
                                             start=(nt == 0), stop=(nt == KT - 1))
                        nc.vector.scalar_tensor_tensor(
                            x2xi[:, c5], psb[:], 2.0, xibm[:, c5],
                            op0=ALU.mult, op1=ALU.subtract)
                    nc.sync.dma_start(xist_d[1:2, :], x1xi[:, :])
                    nc.sync.dma_start(xist_d[2:3, :], x2xi[:, :])

            # ============ PHASE B: per-batch gate pipeline ============
            def phase_b(b_range):
                for b in b_range:
                    bc0 = b * 128
                    hcm = pcmB.tile([U, N], f32r, tag="hcm")
                    nc.sync.dma_start(hcm[:], hcm_d[:, b, :])
                    x1t = pcmA.tile([128, KT, 128], f32r, tag="x1t")
                    nc.sync.dma_start(
                        x1t[:], x1_d[:, bc0:bc0 + 128]
                        .rearrange("(k n) c -> n k c", n=128))
                    xi3 = sm2.tile([3, N], f32r, tag="xi3")
                    nc.sync.dma_start(xi3[:], xist_d[:, b * N:(b + 1) * N])
                    x1cm = pcmA.tile([U, N], f32r, tag="x1cm")
                    for np2 in range(KT // 2):
                        pst = ps2.tile([128, 256], f32, tag="tr")
                        for j in range(2):
                            nc.tensor.transpose(
                                pst[:, j * 128:(j + 1) * 128],
                                x1t[:, np2 * 2 + j, :].bitcast(f32), ident[:])
                        nc.scalar.copy(x1cm[:, np2 * 256:(np2 + 1) * 256], pst[:])
                    x2cm = pcmA.tile([U, N], f32r, tag="x2cm")
                    psx = [psAcc.tile([128, 512], f32, tag="acc",
                                      name=f"bx{b}_{ci}") for ci in range(2)]
                    for nt in range(KT):
                        for ci in range(2):
                            nc.tensor.matmul(
                                psx[ci][:], x1t[:, nt, :],
                                s_sb[:, nt, ci * 512:(ci + 1) * 512],
                                start=(nt == 0), stop=(nt == KT - 1))
                    for ci in range(2):
                        c5 = slice(ci * 512, (ci + 1) * 512)
                        nc.vector.scalar_tensor_tensor(
                            x2cm[:, c5], psx[ci][:], 2.0, hcm[:, c5],
                            op0=ALU.mult, op1=ALU.subtract)
                    # gate W-stage: 4 psums (2 ot x 2 ch), k-outer
                    psw = [[psAcc.tile([128, 512], f32, tag="acc",
                                       name=f"bw{b}_{ot}_{ci}")
                            for ci in range(2)] for ot in range(2)]
                    rhs_k = [hcm, x1cm, x2cm]
                    for ot in range(2):
                        o0 = ot * 128
                        for k in range(3):
                            for ci in range(2):
                                c5 = slice(ci * 512, (ci + 1) * 512)
                                nc.tensor.matmul(psw[ot][ci][:], wgh_k(k, o0),
                                                 rhs_k[k][:, c5],
                                                 start=(k == 0), stop=False)
                        for ci in range(2):
                            c5 = slice(ci * 512, (ci + 1) * 512)
                            nc.tensor.matmul(psw[ot][ci][:],
                                             wb[0:3, WGXI + o0:WGXI + o0 + 128],
                                             xi3[:, c5], start=False, stop=True)
                    for ci in range(2):
                        c5 = slice(ci * 512, (ci + 1) * 512)
                        # r
                        sgr = sm2.tile([128, 512], f32, tag="sg")
                        nc.scalar.activation(sgr[:], psw[0][ci][:], AF.Sigmoid,
                                             bias=wb[:, WGBC:WGBC + 1])
                        rhc = sm1.tile([128, 512], f32r, tag="rhc")
                        nc.vector.tensor_mul(rhc[:], sgr[:], hcm[:, c5])
                        nc.gpsimd.dma_start(rhcm_d[:, b, c5], rhc[:])
                        rht = sm2.tile([128, 4, U], f32r, tag="rht")
                        for tp in range(2):
                            pst = ps2.tile([128, 256], f32, tag="tr")
                            for j in range(2):
                                nc.tensor.transpose(
                                    pst[:, j * 128:(j + 1) * 128],
                                    rhc[:, (tp * 2 + j) * 128:
                                        (tp * 2 + j + 1) * 128]
                                    .bitcast(f32), ident[:])
                            nc.vector.tensor_copy(
                                rht[:, tp * 2:tp * 2 + 2, :]
                                .rearrange("p t u -> p (t u)"), pst[:])
                        m0 = ci * 512
                        nc.sync.dma_start(
                            rhnm_d[m0:m0 + 512, b, :]
                            .rearrange("(t n) u -> n t u", t=4), rht[:])
                        # u
                        sgu = sm2.tile([128, 512], f32, tag="sg")
                        nc.scalar.activation(sgu[:], psw[1][ci][:], AF.Sigmoid,
                                             bias=wb[:, WGBC + 1:WGBC + 2])
                        nc.gpsimd.dma_start(ucm_d[:, b, c5], sgu[:])

            # ================= PHASE C: y1 = S @ rh =================
            def phase_c(g):
                gw = HW // 2
                rh0 = st0.tile([128, KT, gw], f32r, tag="big",
                               name=f"rh0_{g}")
                for kt in range(KT):
                    eng = nc.sync if kt % 2 == 0 else nc.scalar
                    eng.dma_start(
                        rh0[:, kt, :],
                        rhnm_d[kt * 128:(kt + 1) * 128,
                               g * (BC // 2):(g + 1) * (BC // 2), :]
                        .rearrange("n b u -> n (b u)"))
                s_apply(rh0, gw, y1_d, dst_c0=g * gw, pfx=f"c{g}")

            # ============ PHASE D: cand + new_h + proj ============
            def phase_d(b_range):
                for b in b_range:
                    bc0 = b * 128
                    hcm = pcmB.tile([U, N], f32r, tag="hcm")
                    nc.sync.dma_start(hcm[:], hcm_d[:, b, :])
                    rhcm = pcmC.tile([U, N], f32r, tag="rhcm")
                    nc.sync.dma_start(rhcm[:], rhcm_d[:, b, :])
                    usb = pcmC.tile([U, N], f32, tag="usb")
                    nc.sync.dma_start(usb[:], ucm_d[:, b, :])
                    y1t = pcmA.tile([128, KT, 128], f32r, tag="x1t")
                    nc.sync.dma_start(
                        y1t[:], y1_d[:, bc0:bc0 + 128]
                        .rearrange("(k n) c -> n k c", n=128))
                    xi3 = sm2.tile([3, N], f32r, tag="xi3")
                    nc.sync.dma_start(xi3[:], xist_d[:, b * N:(b + 1) * N])
                    y1cm = pcmA.tile([U, N], f32r, tag="x1cm")
                    for np2 in range(KT // 2):
                        pst = ps2.tile([128, 256], f32, tag="tr")
                        for j in range(2):
                            nc.tensor.transpose(
                                pst[:, j * 128:(j + 1) * 128],
                                y1t[:, np2 * 2 + j, :].bitcast(f32), ident[:])
                        nc.scalar.copy(y1cm[:, np2 * 256:(np2 + 1) * 256], pst[:])
                    y2cm = pcmA.tile([U, N], f32r, tag="x2cm")
                    psx = [psAcc.tile([128, 512], f32, tag="acc",
                                      name=f"dx{b}_{ci}") for ci in range(2)]
                    for nt in range(KT):
                        for ci in range(2):
                            nc.tensor.matmul(
                                psx[ci][:], y1t[:, nt, :],
                                s_sb[:, nt, ci * 512:(ci + 1) * 512],
                                start=(nt == 0), stop=(nt == KT - 1))
                    for ci in range(2):
                        c5 = slice(ci * 512, (ci + 1) * 512)
                        nc.vector.scalar_tensor_tensor(
                            y2cm[:, c5], psx[ci][:], 2.0, rhcm[:, c5],
                            op0=ALU.mult, op1=ALU.subtract)
                    psw = [psAcc.tile([128, 512], f32, tag="acc",
                                      name=f"dw{b}_{ci}") for ci in range(2)]
                    rhs_k = [rhcm, y1cm, y2cm]
                    for k in range(3):
                        for ci in range(2):
                            c5 = slice(ci * 512, (ci + 1) * 512)
                            nc.tensor.matmul(psw[ci][:], wch_k(k),
                                             rhs_k[k][:, c5],
                                             start=(k == 0), stop=False)
                    for ci in range(2):
                        c5 = slice(ci * 512, (ci + 1) * 512)
                        nc.tensor.matmul(psw[ci][:], wb[0:3, WCXI:WCXI + 128],
                                         xi3[:, c5], start=False, stop=True)
                    for ci in range(2):
                        c5 = slice(ci * 512, (ci + 1) * 512)
                        t0 = b * N + ci * 512
                        # z1 = u*h computed before tanh lands
                        z1 = sm1.tile([128, 512], f32, tag="rhc")
                        nc.vector.tensor_mul(z1[:], usb[:, c5], hcm[:, c5])
                        csb = sm2.tile([128, 512], f32, tag="sg")
                        nc.scalar.activation(csb[:], psw[ci][:], AF.Tanh,
                                             bias=wb[:, WCBC:WCBC + 1])
                        # new_h = z1 - (u-1)*c
                        t2 = sm2.tile([128, 512], f32, tag="sg")
                        nc.vector.scalar_tensor_tensor(
                            t2[:], usb[:, c5], 1.0, csb[:],
                            op0=ALU.subtract, op1=ALU.mult)
                        nh = sm1.tile([128, 512], f32r, tag="nh")
                        nc.vector.tensor_sub(nh[:], z1[:], t2[:])
                        nc.gpsimd.dma_start(nh_d[:, b, c5], nh[:])
                        psp = ps2.tile([1, 512], f32, tag="sml")
                        nc.tensor.matmul(psp[:], wb[:, WPW0:WPW0 + 1], nh[:],
                                         start=True, stop=True)
                        po = sm2.tile([1, 512], f32, tag="po")
                        nc.scalar.activation(po[:], psp[:], AF.Identity,
                                             bias=wb[0:1, WPB:WPB + 1])
                        nc.gpsimd.dma_start(proj_d[:, t0:t0 + 512], po[:])

            with nc.named_scope("phaseBCD"):
                phase_b(range(0, BC // 2))
                phase_b(range(BC // 2, BC))
                phase_c(0)
                phase_c(1)
                phase_d(range(0, BC // 2))
                phase_d(range(BC // 2, BC))

    nc.compile()
    return nc


def _prep(inputs, hidden_state, support, gate_w, gate_b, cand_w, cand_b,
          proj_w, proj_b):
    """Host-side sharding + layout prep. Returns list of per-core in_maps."""
    f = np.float32
    S_T = np.ascontiguousarray(np.asarray(support, f).T)
    h3 = np.asarray(hidden_state, f)[0].reshape(B, N, U)
    xi = np.asarray(inputs, f).reshape(B, N)
    gw = np.asarray(gate_w, f)
    cw = np.asarray(cand_w, f)
    # W rows: flat col index c*3+k ; c=0 is xi, c=1+j is hidden channel j
    hid_rows = 3 * (1 + np.arange(U))
    shared = dict(
        s_t=S_T,
        wgh=np.ascontiguousarray(np.stack([gw[hid_rows + k] for k in range(3)])),
        wgxi=np.ascontiguousarray(gw[0:3]),
        gb=np.asarray(gate_b, f).reshape(1, 2 * U),
        wch=np.ascontiguousarray(np.stack([cw[hid_rows + k] for k in range(3)])),
        wcxi=np.ascontiguousarray(cw[0:3]),
        cb=np.asarray(cand_b, f).reshape(1, U),
        pw=np.asarray(proj_w, f).reshape(U, 1),
        pb=np.asarray(proj_b, f).reshape(1, 1),
    )
    maps = []
    for c in range(NCORES):
        b0 = c * BC
        hs = h3[b0:b0 + BC]                                       # [BC, N, U]
        h_nm = np.ascontiguousarray(hs.transpose(1, 0, 2)).reshape(N, BC * U)
        h_cm = np.ascontiguousarray(hs.transpose(2, 0, 1))        # [U, BC, N]
        xib = np.ascontiguousarray(xi[b0:b0 + BC])                # [BC, N]
        maps.append(dict(
            shared,
            h_nm=h_nm, h_cm=h_cm,
            xi_nm=np.ascontiguousarray(xib.T),
            xi_bm=xib,
        ))
    return maps


def _gather(results):
    out = np.empty((B, N * OD), np.float32)
    new_h = np.empty((B, N * U), np.float32)
    for c in range(NCORES):
        b0 = c * BC
        r = results[c]
        out[b0:b0 + BC] = r["out_proj"].reshape(BC, N)
        new_h[b0:b0 + BC] = r["nh_cm"].transpose(1, 2, 0).reshape(BC, N * U)
    return out, new_h[None]


def kernel(**inputs):
    from concourse.bass_utils import run_bass_kernel_spmd

    if "nc" not in _cache:
        _cache["nc"] = _build()
    nc = _cache["nc"]
    maps = _prep(**inputs)
    res = run_bass_kernel_spmd(nc, maps, core_ids=list(range(NCORES)),
                               trace=False)
    _cache["last_res"] = res
    return _gather(res.results)


# revision 20
# speedup vs baseline: 1.4138x; 1.0046x over previous
"""DCRNN decoder cell (gnn_message_passing) on 8 Trainium2 cores.

Data-parallel over batch B=128 -> 16 batches/core; support matrix and
weights replicated; no collectives.

Per core (Bc=16):
  x0 = concat([xi, h]); x1 = S x0; x2 = 2 S x1 - x0     (Chebyshev)
  gate = sigmoid(sum_k Xk @ Wg_k + bg) -> r, u
  rh = r*h; y1 = S rh; y2 = 2 S y1 - rh
  c = tanh(sum_k Yk @ Wc_k + bc)
  new_h = u h + (1-u) c ; out = new_h @ pw + pb

Layouts: node-major [n, (b,u)] for Chebyshev; channel-major [u, (b,n)]
for the W-stages (out_T[o,tok] = sum_c Wk[c,o] Xk_cm[c,tok], bias and
the xi channel folded in as K=1 / K=3 matmuls).  x2_cm is computed
directly as (2 S x1 - x0)_cm via lhsT=x1_nm strips, rhs=S^T: the matmul
performs the S-apply and the cm-transpose at once.  x1_cm comes from PE
transposes of x1_nm strips.  All matmuls in float32r (full PE rate).
"""
import numpy as np

import concourse.bacc as bacc
import concourse.tile as tile
from concourse import mybir
from concourse.masks import make_identity

f32 = mybir.dt.float32
f32r = mybir.dt.float32r
AF = mybir.ActivationFunctionType
ALU = mybir.AluOpType

B, N, U, OD = 128, 1024, 128, 1
NCORES = 8
BC = B // NCORES            # 16 batches per core
KT = N // 128               # 8 node tiles
TOK = BC * N                # 16384 tokens per core
HW = BC * U                 # 2048 hidden cols
XW = HW + BC                # 2064 = hidden + xi cols

# packed small-weights tile column offsets (f32r [128, 1792])
WGH0 = 0            # [128, 3*256] gate hidden weights, c-major
WCH0 = 768          # [128, 3*128] cand hidden weights
WPW0 = WCH0 + 384   # [128, 1] proj weight
WROW = WPW0 + 1     # row-tensors region (partitions 0..2):
#   wgxi [3, 256] @ WROW, wcxi [3, 128] @ +256, gb [1, 256] @ +384 (p0),
#   cb [1,128] @ +640 (p0), pb [1,1] @ +768 (p0), ones [1,512] @ +769 (p0)
WGXI = WROW
WCXI = WROW + 256
WGBC = WCXI + 128   # [128, 2] gate bias as columns (r, u)
WCBC = WGBC + 2     # [128, 1] cand bias column
WPB = WCBC + 1      # [1, 1] proj bias
WBIGW = WPB + 1

_cache = {}


def _build():
    nc = bacc.Bacc("TRN2", target_bir_lowering=False, debug=False)

    s_d = nc.dram_tensor("s_t", [N, N], f32r, kind="ExternalInput").ap()
    hnm_d = nc.dram_tensor("h_nm", [N, BC * U], f32r, kind="ExternalInput").ap()
    hcm_d = nc.dram_tensor("h_cm", [U, BC, N], f32r, kind="ExternalInput").ap()
    xinm_d = nc.dram_tensor("xi_nm", [N, BC], f32r, kind="ExternalInput").ap()
    xibm_d = nc.dram_tensor("xi_bm", [BC, N], f32r, kind="ExternalInput").ap()
    wgh_d = nc.dram_tensor("wgh", [3, U, 2 * U], f32r, kind="ExternalInput").ap()
    wgxi_d = nc.dram_tensor("wgxi", [3, 2 * U], f32r, kind="ExternalInput").ap()
    gb_d = nc.dram_tensor("gb", [1, 2 * U], f32r, kind="ExternalInput").ap()
    wch_d = nc.dram_tensor("wch", [3, U, U], f32r, kind="ExternalInput").ap()
    wcxi_d = nc.dram_tensor("wcxi", [3, U], f32r, kind="ExternalInput").ap()
    cb_d = nc.dram_tensor("cb", [1, U], f32r, kind="ExternalInput").ap()
    pw_d = nc.dram_tensor("pw", [U, 1], f32r, kind="ExternalInput").ap()
    pb_d = nc.dram_tensor("pb", [1, 1], f32r, kind="ExternalInput").ap()

    proj_d = nc.dram_tensor("out_proj", [1, TOK], f32, kind="ExternalOutput").ap()
    nh_d = nc.dram_tensor("nh_cm", [U, BC, N], f32r, kind="ExternalOutput").ap()

    x1_d = nc.dram_tensor("x1_nm", [N, XW], f32r, kind="Internal").ap()
    y1_d = nc.dram_tensor("y1_nm", [N, HW], f32r, kind="Internal").ap()
    rhnm_d = nc.dram_tensor("rh_nm", [N, BC, U], f32r, kind="Internal").ap()
    rhcm_d = nc.dram_tensor("rh_cm", [U, BC, N], f32r, kind="Internal").ap()
    ucm_d = nc.dram_tensor("u_cm", [U, BC, N], f32, kind="Internal").ap()
    xist_d = nc.dram_tensor("xi_stack", [3, TOK], f32r, kind="Internal").ap()

    with tile.TileContext(nc) as tc:
        with (
            tc.tile_pool(name="consts", bufs=1) as consts,
            tc.tile_pool(name="stage1", bufs=1) as st1,
            tc.tile_pool(name="stage0", bufs=1) as st0,
            tc.tile_pool(name="pcmA", bufs=2) as pcmA,
            tc.tile_pool(name="pcmB", bufs=2) as pcmB,
            tc.tile_pool(name="pcmC", bufs=2) as pcmC,
            tc.tile_pool(name="sm2", bufs=2) as sm2,
            tc.tile_pool(name="sm1", bufs=2) as sm1,
            tc.tile_pool(name="ps", bufs=4, space="PSUM") as psAcc,
            tc.tile_pool(name="ps2", bufs=2, space="PSUM") as ps2,
        ):
            # ---------------- constants ----------------
            ident = consts.tile([128, 128], f32)
            make_identity(nc, ident)
            s_sb = consts.tile([128, KT, N], f32r)
            for kt in range(KT):
                nc.sync.dma_start(s_sb[:, kt, :], s_d[kt * 128:(kt + 1) * 128, :])
            wb = consts.tile([128, WBIGW], f32r)
            nc.sync.dma_start(
                wb[:, WGH0:WGH0 + 768].rearrange("c (k o) -> c k o", k=3),
                wgh_d[:, :, :].rearrange("k c o -> c k o"))
            nc.sync.dma_start(
                wb[:, WCH0:WCH0 + 384].rearrange("c (k o) -> c k o", k=3),
                wch_d[:, :, :].rearrange("k c o -> c k o"))
            nc.sync.dma_start(wb[:, WPW0:WPW0 + 1], pw_d[:, :])
            nc.sync.dma_start(wb[0:3, WGXI:WGXI + 256], wgxi_d[:, :])
            nc.sync.dma_start(wb[0:3, WCXI:WCXI + 128], wcxi_d[:, :])
            nc.sync.dma_start(wb[:, WGBC:WGBC + 2],
                              gb_d[:, :].rearrange("one (t c) -> c (one t)", t=2))
            nc.sync.dma_start(wb[:, WCBC:WCBC + 1],
                              cb_d[:, :].rearrange("one c -> c one"))
            nc.sync.dma_start(wb[0:1, WPB:WPB + 1], pb_d[:, :])

            def wgh_k(k, o0):
                return wb[:, WGH0 + k * 256 + o0:WGH0 + k * 256 + o0 + 128]

            def wch_k(k):
                return wb[:, WCH0 + k * 128:WCH0 + (k + 1) * 128]

            # -------- helper: node-major S-apply, resident source --------
            def s_apply(src_res, width, dst_dram, dst_c0=0, pfx=""):
                """dst_dram[mt-block, :width] = S @ src_res ([128, KT, width]).
                mt/kt-outer so each stationary S-tile serves all chunks."""
                nch = width // 512
                tail = width - nch * 512
                for mt in range(KT):
                    pss = [(psAcc if ci < 2 else ps2)
                           .tile([128, 512], f32,
                                 tag=("acc" if ci < 2 else "sml"),
                                 name=f"apl{pfx}{mt}_{ci}")
                           for ci in range(nch)]
                    pxi = (ps2.tile([128, 128], f32, tag="tr",
                                    name=f"apx{pfx}{mt}")
                           if tail else None)
                    for kt in range(KT):
                        for ci in range(nch):
                            nc.tensor.matmul(
                                pss[ci][:], s_sb[:, kt, mt * 128:(mt + 1) * 128],
                                src_res[:, kt, ci * 512:(ci + 1) * 512],
                                start=(kt == 0), stop=(kt == KT - 1))
                        if tail:
                            nc.tensor.matmul(
                                pxi[:, 0:tail],
                                s_sb[:, kt, mt * 128:(mt + 1) * 128],
                                src_res[:, kt, nch * 512:width],
                                start=(kt == 0), stop=(kt == KT - 1))
                    for ci in range(nch):
                        so = sm2.tile([128, 512], f32r, tag="so",
                                      name=f"so{pfx}{mt}_{ci}")
                        nc.vector.tensor_copy(so[:], pss[ci][:])
                        nc.sync.dma_start(
                            dst_dram[mt * 128:(mt + 1) * 128,
                                     dst_c0 + ci * 512:dst_c0 + (ci + 1) * 512],
                            so[:])
                    if tail:
                        sot = sm2.tile([128, tail], f32r, tag="po",
                                       name=f"sot{pfx}{mt}")
                        nc.vector.tensor_copy(sot[:], pxi[:, 0:tail])
                        nc.sync.dma_start(
                            dst_dram[mt * 128:(mt + 1) * 128,
                                     dst_c0 + nch * 512:dst_c0 + width],
                            sot[:])

            # ================= PHASE A: x1 = S @ x0 =================
            with nc.named_scope("phaseA"):
                x0 = st0.tile([128, KT, XW], f32r, tag="big")
                for kt in range(KT):
                    eng = nc.sync if kt % 2 == 0 else nc.scalar
                    eng.dma_start(x0[:, kt, 0:HW],
                                  hnm_d[kt * 128:(kt + 1) * 128, :])
                    eng.dma_start(x0[:, kt, HW:XW],
                                  xinm_d[kt * 128:(kt + 1) * 128, :])
                s_apply(x0, XW, x1_d)

                # xi chebyshev rows -> DRAM xi_stack
                with tc.tile_pool(name="xip", bufs=1) as xip:
                    xibm = xip.tile([BC, N], f32r)
                    nc.sync.dma_start(xibm[:], xibm_d[:, :])
                    nc.sync.dma_start(xist_d[0:1, :], xibm[:, :])
                    x1xit = xip.tile([128, KT, BC], f32r)
                    nc.sync.dma_start(
                        x1xit[:], x1_d[:, HW:XW].rearrange("(k n) c -> n k c",
                                                           n=128))
                    x1xi = xip.tile([BC, N], f32r)
                    x2xi = xip.tile([BC, N], f32r)
                    for ch in range(2):
                        c5 = slice(ch * 512, (ch + 1) * 512)
                        ps = ps2.tile([BC, 512], f32, tag="sml")
                        for kt in range(KT):
                            nc.tensor.matmul(ps[:], x0[:, kt, HW:XW],
                                             s_sb[:, kt, c5],
                                             start=(kt == 0), stop=(kt == KT - 1))
                        nc.vector.tensor_copy(x1xi[:, c5], ps[:])
                        psb = ps2.tile([BC, 512], f32, tag="sml")
                        for nt in range(KT):
                            nc.tensor.matmul(psb[:], x1xit[:, nt, :],
                                             s_sb[:, nt, c5],
                                             start=(nt == 0), stop=(nt == KT - 1))
                        nc.vector.scalar_tensor_tensor(
                            x2xi[:, c5], psb[:], 2.0, xibm[:, c5],
                            op0=ALU.mult, op1=ALU.subtract)
                    nc.sync.dma_start(xist_d[1:2, :], x1xi[:, :])
                    nc.sync.dma_start(xist_d[2:3, :], x2xi[:, :])

            # ============ PHASE B: per-batch gate pipeline ============
            def phase_b(b_range):
                for b in b_range:
                    bc0 = b * 128
                    hcm = pcmB.tile([U, N], f32r, tag="hcm")
                    nc.sync.dma_start(hcm[:], hcm_d[:, b, :])
                    x1t = pcmA.tile([128, KT, 128], f32r, tag="x1t")
                    nc.sync.dma_start(
                        x1t[:], x1_d[:, bc0:bc0 + 128]
                        .rearrange("(k n) c -> n k c", n=128))
                    xi3 = sm2.tile([3, N], f32r, tag="xi3")
                    nc.sync.dma_start(xi3[:], xist_d[:, b * N:(b + 1) * N])
                    x1cm = pcmA.tile([U, N], f32r, tag="x1cm")
                    for np2 in range(KT // 2):
                        pst = ps2.tile([128, 256], f32, tag="tr")
                        for j in range(2):
                            nc.tensor.transpose(
                                pst[:, j * 128:(j + 1) * 128],
                                x1t[:, np2 * 2 + j, :].bitcast(f32), ident[:])
                        nc.scalar.copy(x1cm[:, np2 * 256:(np2 + 1) * 256], pst[:])
                    x2cm = pcmA.tile([U, N], f32r, tag="x2cm")
                    psx = [psAcc.tile([128, 512], f32, tag="acc",
                                      name=f"bx{b}_{ci}") for ci in range(2)]
                    for nt in range(KT):
                        for ci in range(2):
                            nc.tensor.matmul(
                                psx[ci][:], x1t[:, nt, :],
                                s_sb[:, nt, ci * 512:(ci + 1) * 512],
                                start=(nt == 0), stop=(nt == KT - 1))
                    for ci in range(2):
                        c5 = slice(ci * 512, (ci + 1) * 512)
                        nc.vector.scalar_tensor_tensor(
                            x2cm[:, c5], psx[ci][:], 2.0, hcm[:, c5],
                            op0=ALU.mult, op1=ALU.subtract)
                    # gate W-stage: 4 psums (2 ot x 2 ch), k-outer
                    psw = [[psAcc.tile([128, 512], f32, tag="acc",
                                       name=f"bw{b}_{ot}_{ci}")
                            for ci in range(2)] for ot in range(2)]
                    rhs_k = [hcm, x1cm, x2cm]
                    for ot in range(2):
                        o0 = ot * 128
                        for k in range(3):
                            for ci in range(2):
                                c5 = slice(ci * 512, (ci + 1) * 512)
                                nc.tensor.matmul(psw[ot][ci][:], wgh_k(k, o0),
                                                 rhs_k[k][:, c5],
                                                 start=(k == 0), stop=False)
                        for ci in range(2):
                            c5 = slice(ci * 512, (ci + 1) * 512)
                            nc.tensor.matmul(psw[ot][ci][:],
                                             wb[0:3, WGXI + o0:WGXI + o0 + 128],
                                             xi3[:, c5], start=False, stop=True)
                    for ci in range(2):
                        c5 = slice(ci * 512, (ci + 1) * 512)
                        # r
                        sgr = sm2.tile([128, 512], f32, tag="sg")
                        nc.scalar.activation(sgr[:], psw[0][ci][:], AF.Sigmoid,
                                             bias=wb[:, WGBC:WGBC + 1])
                        rhc = sm1.tile([128, 512], f32r, tag="rhc")
                        nc.vector.tensor_mul(rhc[:], sgr[:], hcm[:, c5])
                        nc.gpsimd.dma_start(rhcm_d[:, b, c5], rhc[:])
                        rht = sm2.tile([128, 4, U], f32r, tag="rht")
                        for tp in range(2):
                            pst = ps2.tile([128, 256], f32, tag="tr")
                            for j in range(2):
                                nc.tensor.transpose(
                                    pst[:, j * 128:(j + 1) * 128],
                                    rhc[:, (tp * 2 + j) * 128:
                                        (tp * 2 + j + 1) * 128]
                                    .bitcast(f32), ident[:])
                            nc.vector.tensor_copy(
                                rht[:, tp * 2:tp * 2 + 2, :]
                                .rearrange("p t u -> p (t u)"), pst[:])
                        m0 = ci * 512
                        nc.sync.dma_start(
                            rhnm_d[m0:m0 + 512, b, :]
                            .rearrange("(t n) u -> n t u", t=4), rht[:])
                        # u
                        sgu = sm2.tile([128, 512], f32, tag="sg")
                        nc.scalar.activation(sgu[:], psw[1][ci][:], AF.Sigmoid,
                                             bias=wb[:, WGBC + 1:WGBC + 2])
                        nc.gpsimd.dma_start(ucm_d[:, b, c5], sgu[:])

            # ================= PHASE C: y1 = S @ rh =================
            def phase_c(g):
                gw = HW // 2
                rh0 = st0.tile([128, KT, gw], f32r, tag="big",
                               name=f"rh0_{g}")
                for kt in range(KT):
                    eng = nc.sync if kt % 2 == 0 else nc.scalar
                    eng.dma_start(
                        rh0[:, kt, :],
                        rhnm_d[kt * 128:(kt + 1) * 128,
                               g * (BC // 2):(g + 1) * (BC // 2), :]
                        .rearrange("n b u -> n (b u)"))
                s_apply(rh0, gw, y1_d, dst_c0=g * gw, pfx=f"c{g}")

            # ============ PHASE D: cand + new_h + proj ============
            def phase_d(b_range):
                for b in b_range:
                    bc0 = b * 128
                    hcm = pcmB.tile([U, N], f32r, tag="hcm")
                    nc.sync.dma_start(hcm[:], hcm_d[:, b, :])
                    rhcm = pcmC.tile([U, N], f32r, tag="rhcm")
                    nc.sync.dma_start(rhcm[:], rhcm_d[:, b, :])
                    usb = pcmC.tile([U, N], f32, tag="usb")
                    nc.sync.dma_start(usb[:], ucm_d[:, b, :])
                    y1t = pcmA.tile([128, KT, 128], f32r, tag="x1t")
                    nc.sync.dma_start(
                        y1t[:], y1_d[:, bc0:bc0 + 128]
                        .rearrange("(k n) c -> n k c", n=128))
                    xi3 = sm2.tile([3, N], f32r, tag="xi3")
                    nc.sync.dma_start(xi3[:], xist_d[:, b * N:(b + 1) * N])
                    y1cm = pcmA.tile([U, N], f32r, tag="x1cm")
                    for np2 in range(KT // 2):
                        pst = ps2.tile([128, 256], f32, tag="tr")
                        for j in range(2):
                            nc.tensor.transpose(
                                pst[:, j * 128:(j + 1) * 128],
                                y1t[:, np2 * 2 + j, :].bitcast(f32), ident[:])
                        nc.scalar.copy(y1cm[:, np2 * 256:(np2 + 1) * 256], pst[:])
                    y2cm = pcmA.tile([U, N], f32r, tag="x2cm")
                    psx = [psAcc.tile([128, 512], f32, tag="acc",
                                      name=f"dx{b}_{ci}") for ci in range(2)]
                    for nt in range(KT):
                        for ci in range(2):
                            nc.tensor.matmul(
                                psx[ci][:], y1t[:, nt, :],
                                s_sb[:, nt, ci * 512:(ci + 1) * 512],
                                start=(nt == 0), stop=(nt == KT - 1))
                    for ci in range(2):
                        c5 = slice(ci * 512, (ci + 1) * 512)
                        nc.vector.scalar_tensor_tensor(
                            y2cm[:, c5], psx[ci][:], 2.0, rhcm[:, c5],
                            op0=ALU.mult, op1=ALU.subtract)
                    psw = [psAcc.tile([128, 512], f32, tag="acc",
                                      name=f"dw{b}_{ci}") for ci in range(2)]
                    rhs_k = [rhcm, y1cm, y2cm]
                    for k in range(3):
                        for ci in range(2):
                            c5 = slice(ci * 512, (ci + 1) * 512)
                            nc.tensor.matmul(psw[ci][:], wch_k(k),
                                             rhs_k[k][:, c5],
                                             start=(k == 0), stop=False)
                    for ci in range(2):
                        c5 = slice(ci * 512, (ci + 1) * 512)
                        nc.tensor.matmul(psw[ci][:], wb[0:3, WCXI:WCXI + 128],
                                         xi3[:, c5], start=False, stop=True)
                    for ci in range(2):
                        c5 = slice(ci * 512, (ci + 1) * 512)
                        t0 = b * N + ci * 512
                        # z1 = u*h computed before tanh lands
                        z1 = sm1.tile([128, 512], f32, tag="rhc")
                        nc.vector.tensor_mul(z1[:], usb[:, c5], hcm[:, c5])
                        csb = sm2.tile([128, 512], f32, tag="sg")
                        nc.scalar.activation(csb[:], psw[ci][:], AF.Tanh,
                                             bias=wb[:, WCBC:WCBC + 1])
                        # new_h = z1 - (u-1)*c
                        t2 = sm2.tile([128, 512], f32, tag="sg")
                        nc.vector.scalar_tensor_tensor(
                            t2[:], usb[:, c5], 1.0, csb[:],
                            op0=ALU.subtract, op1=ALU.mult)
                        nh = sm1.tile([128, 512], f32r, tag="nh")
                        nc.vector.tensor_sub(nh[:], z1[:], t2[:])
                        nc.gpsimd.dma_start(nh_d[:, b, c5], nh[:])
                        psp = ps2.tile([1, 512], f32, tag="sml")
                        nc.tensor.matmul(psp[:], wb[:, WPW0:WPW0 + 1], nh[:],
                                         start=True, stop=True)
                        po = sm2.tile([1, 512], f32, tag="po")
                        nc.scalar.activation(po[:], psp[:], AF.Identity,
                                             bias=wb[0:1, WPB:WPB + 1])
                        nc.gpsimd.dma_start(proj_d[:, t0:t0 + 512], po[:])

            with nc.named_scope("phaseBCD"):
                phase_b(range(0, BC // 2))
                phase_b(range(BC // 2, BC))
                phase_c(0)
                phase_c(1)
                phase_d(range(0, BC // 2))
                phase_d(range(BC // 2, BC))

    nc.compile()
    return nc


def _prep(inputs, hidden_state, support, gate_w, gate_b, cand_w, cand_b,
          proj_w, proj_b):
    """Host-side sharding + layout prep. Returns list of per-core in_maps."""
    f = np.float32
    S_T = np.ascontiguousarray(np.asarray(support, f).T)
    h3 = np.asarray(hidden_state, f)[0].reshape(B, N, U)
    xi = np.asarray(inputs, f).reshape(B, N)
    gw = np.asarray(gate_w, f)
    cw = np.asarray(cand_w, f)
    # W rows: flat col index c*3+k ; c=0 is xi, c=1+j is hidden channel j
    hid_rows = 3 * (1 + np.arange(U))
    shared = dict(
        s_t=S_T,
        wgh=np.ascontiguousarray(np.stack([gw[hid_rows + k] for k in range(3)])),
        wgxi=np.ascontiguousarray(gw[0:3]),
        gb=np.asarray(gate_b, f).reshape(1, 2 * U),
        wch=np.ascontiguousarray(np.stack([cw[hid_rows + k] for k in range(3)])),
        wcxi=np.ascontiguousarray(cw[0:3]),
        cb=np.asarray(cand_b, f).reshape(1, U),
        pw=np.asarray(proj_w, f).reshape(U, 1),
        pb=np.asarray(proj_b, f).reshape(1, 1),
    )
    maps = []
    for c in range(NCORES):
        b0 = c * BC
        hs = h3[b0:b0 + BC]                                       # [BC, N, U]
        h_nm = np.ascontiguousarray(hs.transpose(1, 0, 2)).reshape(N, BC * U)
        h_cm = np.ascontiguousarray(hs.transpose(2, 0, 1))        # [U, BC, N]
        xib = np.ascontiguousarray(xi[b0:b0 + BC])                # [BC, N]
        maps.append(dict(
            shared,
            h_nm=h_nm, h_cm=h_cm,
            xi_nm=np.ascontiguousarray(xib.T),
            xi_bm=xib,
        ))
    return maps


def _gather(results):
    out = np.empty((B, N * OD), np.float32)
    new_h = np.empty((B, N * U), np.float32)
    for c in range(NCORES):
        b0 = c * BC
        r = results[c]
        out[b0:b0 + BC] = r["out_proj"].reshape(BC, N)
        new_h[b0:b0 + BC] = r["nh_cm"].transpose(1, 2, 0).reshape(BC, N * U)
    return out, new_h[None]


def kernel(**inputs):
    from concourse.bass_utils import run_bass_kernel_spmd

    if "nc" not in _cache:
        _cache["nc"] = _build()
    nc = _cache["nc"]
    maps = _prep(**inputs)
    res = run_bass_kernel_spmd(nc, maps, core_ids=list(range(NCORES)),
                               trace=False)
    _cache["last_res"] = res
    return _gather(res.results)
